# revision 3
# baseline (speedup 1.0000x reference)
"""DecoderRNN (3-layer LSTM, H=1024, B=256, L=128 steps) on 8 trn2 NeuronCores.

Strategy: 8-way tensor parallel over hidden units (feature-major layout).
Core k owns hidden units [128k, 128k+128) of every layer, holding the matching
512 gate columns of each weight matrix SBUF-resident for all 128 timesteps.
Activations live transposed (h.T : [hidden, batch]); after each layer-step the
128-row h.T chunk is AllGather'd so every core has the full h.T for the next
matmul. Layers are wavefront-pipelined (layer l does step t at phase t+l) so
each phase depends only on the previous phase's three independent AllGathers.
The time-invariant layer-0 input projection (x_in @ W_ih0 + biases) is
precomputed on the host and injected into PSUM with an identity matmul.
"""
import sys
import os

sys.path.insert(0, "/opt/trn_rl_repo")

import numpy as np

B = 256          # batch
H = 1024         # hidden size
NLAYERS = 3
STEPS = 128      # decoded sequence length
OUT = 204        # output size (68*3)
NUM_CLASSES = 12
IN0 = OUT + NUM_CLASSES
NCORES = 8
HC = H // NCORES          # hidden chunk per core = 128
GC = 4 * HC               # gate columns per core = 512
KT = H // 128             # contraction k-tiles = 8
FCC = 26                  # legacy fc shard width (unused in K-sharded fc)
FCP = 208                 # padded fc output size (>= 204)

MM_DTYPE = os.environ.get("KERNEL_MM_DTYPE", "bfloat16")  # bfloat16|float32r|float32

_BUILD_CACHE = {}


def _build(steps: int):
    """Build + compile the SPMD Bass program (same NEFF for all 8 cores)."""
    import concourse.bass as bass
    import concourse.bacc as bacc
    import concourse.tile as tile
    import concourse.mybir as mybir

    f32 = mybir.dt.float32
    mdt = getattr(mybir.dt, MM_DTYPE)   # dtype of all matmul operands
    AF = mybir.ActivationFunctionType

    nc = bacc.Bacc("TRN2", target_bir_lowering=False, debug=False,
                   num_devices=NCORES)

    # ---- kernel I/O -----------------------------------------------------
    w_names = ["whh0", "wih1", "whh1", "wih2", "whh2"]
    w_ext = {n: nc.dram_tensor(n, [128, KT * GC], mdt, kind="ExternalInput")
             for n in w_names}
    gx0_ext = nc.dram_tensor("gx0", [128, 4 * B], mdt, kind="ExternalInput")
    b_ext = {l: nc.dram_tensor(f"b{l}", [128, 4], f32, kind="ExternalInput")
             for l in (1, 2)}
    fcw_ext = nc.dram_tensor("fcw", [128, FCP], mdt, kind="ExternalInput")
    eye_ext = nc.dram_tensor("eye", [128, 128], mdt, kind="ExternalInput")
    hinit_ext = [nc.dram_tensor(f"h{l}i", [128, KT * B], mdt, kind="ExternalInput")
                 for l in range(NLAYERS)]
    cinit_ext = [nc.dram_tensor(f"c{l}i", [HC, B], f32, kind="ExternalInput")
                 for l in range(NLAYERS)]
    out_ext = nc.dram_tensor("out", [steps, FCP, B], f32, kind="ExternalOutput")

    rg = [list(range(NCORES))]

    with tile.TileContext(nc) as tc:
        with tc.tile_pool(name="const", bufs=1) as cpool, \
             tc.tile_pool(name="work", bufs=2) as wpool, \
             tc.tile_pool(name="psum", bufs=2, space="PSUM") as ppool, \
             tc.tile_pool(name="dram", bufs=2, space="DRAM") as dpool:

            # ---- load constants into SBUF (resident for whole kernel) ----
            w_sb = {}
            for n in w_names:
                t = cpool.tile([128, KT * GC], mdt, name=f"sb_{n}")
                nc.sync.dma_start(t[:], w_ext[n][:])
                w_sb[n] = t
            gx0 = cpool.tile([128, 4 * B], mdt, name="sb_gx0")
            nc.sync.dma_start(gx0[:], gx0_ext[:])
            b_sb = {}
            for l in (1, 2):
                t = cpool.tile([128, 4], f32, name=f"sb_b{l}")
                nc.sync.dma_start(t[:], b_ext[l][:])
                b_sb[l] = t
            fcw = cpool.tile([128, FCP], mdt, name="sb_fcw")
            nc.sync.dma_start(fcw[:], fcw_ext[:])
            eye = cpool.tile([128, 128], mdt, name="sb_eye")
            nc.sync.dma_start(eye[:], eye_ext[:])

            h_buf = []
            c_buf = []
            for l in range(NLAYERS):
                ht = wpool.tile([128, KT * B], mdt, tag=f"h{l}", bufs=3,
                                name=f"h{l}_init")
                nc.sync.dma_start(ht[:], hinit_ext[l][:])
                h_buf.append(ht)
                ct = wpool.tile([HC, B], f32, tag=f"c{l}", name=f"c{l}_init")
                nc.sync.dma_start(ct[:], cinit_ext[l][:])
                c_buf.append(ct)

            w_hh = {0: w_sb["whh0"], 1: w_sb["whh1"], 2: w_sb["whh2"]}
            w_ih = {1: w_sb["wih1"], 2: w_sb["wih2"]}

            def emit_layer(l, p):
                """LSTM layer l consuming h_buf/c_buf state; returns AG out."""
                h_self = h_buf[l]
                h_below = h_buf[l - 1] if l > 0 else None
                # gate order in weights: i,f,g,o ; emit f,i,g,o so the cell
                # can start as early as possible.
                acts = {}
                for g in (1, 0, 2, 3):
                    ps = ppool.tile([HC, B], f32, tag="gates", bufs=6,
                                    name=f"ps_l{l}_p{p}_g{g}")
                    n_mm = (1 if l == 0 else 0) + KT * (1 if l == 0 else 2)
                    idx = 0
                    if l == 0:
                        nc.tensor.matmul(ps[:], eye[:],
                                         gx0[:, g * B:(g + 1) * B],
                                         start=True, stop=(idx == n_mm - 1))
                        idx += 1
                    for kt in range(KT):
                        nc.tensor.matmul(
                            ps[:],
                            w_hh[l][:, (kt * 4 + g) * HC:(kt * 4 + g + 1) * HC],
                            h_self[:, kt * B:(kt + 1) * B],
                            start=(idx == 0), stop=(idx == n_mm - 1))
                        idx += 1
                    if l > 0:
                        for kt in range(KT):
                            nc.tensor.matmul(
                                ps[:],
                                w_ih[l][:, (kt * 4 + g) * HC:(kt * 4 + g + 1) * HC],
                                h_below[:, kt * B:(kt + 1) * B],
                                start=False, stop=(idx == n_mm - 1))
                            idx += 1
                    a = wpool.tile([HC, B], f32, tag="gact", bufs=10,
                                   name=f"act_l{l}_p{p}_g{g}")
                    func = AF.Tanh if g == 2 else AF.Sigmoid
                    bias = b_sb[l][:, g:g + 1] if l > 0 else 0.0
                    nc.scalar.activation(a[:], ps[:], func, bias=bias)
                    acts[g] = a

                t_fc = wpool.tile([HC, B], f32, tag="tfc", bufs=3, name=f"tfc{l}_{p}")
                nc.vector.tensor_mul(t_fc[:], acts[1][:], c_buf[l][:])
                t_ig = wpool.tile([HC, B], f32, tag="tig", bufs=3, name=f"tig{l}_{p}")
                nc.vector.tensor_mul(t_ig[:], acts[0][:], acts[2][:])
                c_new = wpool.tile([HC, B], f32, tag=f"c{l}", name=f"c{l}_p{p}")
                nc.vector.tensor_add(c_new[:], t_fc[:], t_ig[:])
                c_buf[l] = c_new
                th = wpool.tile([HC, B], f32, tag="th", bufs=3, name=f"th{l}_{p}")
                nc.scalar.activation(th[:], c_new[:], AF.Tanh)
                hch = wpool.tile([HC, B], mdt, tag="hch", bufs=6, name=f"hch{l}_{p}")
                nc.vector.tensor_mul(hch[:], acts[3][:], th[:])
                return hch

            def emit_fc(t, hch2):
                # K-sharded fc: partial = fc_W[128k:128k+128, :].T @ h2_chunk
                # (local, no gathered h2 needed); host sums the 8 partials.
                ps1 = ppool.tile([128, B], f32, tag="fc1", bufs=1,
                                 name=f"psfc1_{t}")
                nc.tensor.matmul(ps1[:], fcw[:, 0:128], hch2[:],
                                 start=True, stop=True)
                ps2 = ppool.tile([FCP - 128, B], f32, tag="fc2", bufs=1,
                                 name=f"psfc2_{t}")
                nc.tensor.matmul(ps2[:], fcw[:, 128:FCP], hch2[:],
                                 start=True, stop=True)
                o1 = wpool.tile([128, B], f32, tag="osb1", bufs=2, name=f"o1_{t}")
                nc.vector.tensor_copy(o1[:], ps1[:])
                o2 = wpool.tile([FCP - 128, B], f32, tag="osb2", bufs=2,
                                name=f"o2_{t}")
                nc.vector.tensor_copy(o2[:], ps2[:])
                # out stores go on the gpsimd SWDGE queue: the sync queue is
                # reserved for AG-landing DMAs (the phase-critical path).
                nc.gpsimd.dma_start(out_ext[t, 0:128, :], o1[:])
                nc.gpsimd.dma_start(out_ext[t, 128:FCP, :], o2[:])

            def emit_ag(layers_hch, p):
                """AllGather the given layers' chunks (merged when >1)."""
                nl = len(layers_hch)
                tag = "ag" + "".join(str(l) for l in layers_hch)
                agi = dpool.tile([nl * HC, B], mdt, tag=f"i{tag}",
                                 name=f"agi{tag}_p{p}")
                for i, (l, hch) in enumerate(layers_hch.items()):
                    # staging on the gpsimd SWDGE queue — it precedes the
                    # AG trigger there, keeping both sync and scalar free
                    nc.gpsimd.dma_start(agi[i * HC:(i + 1) * HC, :], hch[:])
                ago = dpool.tile([nl * H, B], mdt, tag=f"o{tag}",
                                 addr_space="Shared", name=f"ago{tag}_p{p}")
                nc.gpsimd.collective_compute(
                    "AllGather", mybir.AluOpType.bypass, replica_groups=rg,
                    ins=[agi[:].opt()], outs=[ago[:].opt()])
                # land gathers into fresh SBUF h tiles on the sync HWDGE
                # queue, which carries ONLY landings: the wait-for-AG then
                # never head-of-line blocks activations (scalar) or bounce
                # DMAs (gpsimd), and the landing issues the moment the AG
                # completes — it is the phase-critical chain.
                ago_v = ago.rearrange("(j l p) f -> l p j f", l=nl, p=HC)
                for i, l in enumerate(layers_hch):
                    h_new = wpool.tile([128, KT * B], mdt, tag=f"h{l}", bufs=3,
                                       name=f"h{l}_p{p}")
                    nc.sync.dma_start(
                        h_new.rearrange("p (j f) -> p j f", j=KT), ago_v[i])
                    h_buf[l] = h_new

            # ---- wavefront over phases -----------------------------------
            for p in range(steps + NLAYERS - 1):
                hchs = {}
                if p <= steps - 1:
                    hchs[0] = emit_layer(0, p)
                if 1 <= p <= steps:
                    hchs[1] = emit_layer(1, p)
                if 2 <= p <= steps + 1:
                    hchs[2] = emit_layer(2, p)
                    emit_fc(p - 2, hchs[2])
                # solo per-layer AGs: each fires right after its cell so
                # next-phase consumers unblock as early as possible
                for l, hch in hchs.items():
                    emit_ag({l: hch}, p)

    nc.compile()
    return nc


def _get_nc(steps: int):
    key = (steps, MM_DTYPE)
    if key not in _BUILD_CACHE:
        _BUILD_CACHE[key] = _build(steps)
    return _BUILD_CACHE[key]


def _prep_inputs(inputs, W_ih0, W_hh0, b_ih0, b_hh0, W_ih1, W_hh1, b_ih1, b_hh1,
                 W_ih2, W_hh2, b_ih2, b_hh2, fc_W, fc_b, inh_W, inh_b,
                 inc_W, inc_b, labels):
    """Host-side shard prep: all arrays already in the on-device layout."""
    f32 = np.float32
    frame = np.asarray(inputs, f32).reshape(B, OUT)
    onehot = np.zeros((B, NUM_CLASSES), f32)
    onehot[:, int(labels)] = 1.0
    x_in = np.concatenate([frame, onehot], axis=1)                  # [B, 216]

    h0_all = frame @ np.asarray(inh_W, f32) + np.asarray(inh_b, f32)  # [B, 3H]
    c0_all = frame @ np.asarray(inc_W, f32) + np.asarray(inc_b, f32)
    gx0_full = (x_in @ np.asarray(W_ih0, f32)
                + np.asarray(b_ih0, f32) + np.asarray(b_hh0, f32))  # [B, 4H]

    fcw_pad = np.zeros((H, FCP), f32)
    fcw_pad[:, :OUT] = np.asarray(fc_W, f32)
    fcb_pad = np.zeros(FCP, f32)
    fcb_pad[:OUT] = np.asarray(fc_b, f32)

    def pack_w(Wfull, k):
        Wfull = np.asarray(Wfull, f32)
        cols = np.concatenate(
            [Wfull[:, g * H + k * HC: g * H + (k + 1) * HC] for g in range(4)],
            axis=1)                                                  # [K, 512]
        kt = Wfull.shape[0] // 128
        return np.ascontiguousarray(
            cols.reshape(kt, 128, 4, HC).transpose(1, 0, 2, 3).reshape(128, kt * GC))

    def pack_bias(bi, bh, k):
        s = np.asarray(bi, f32) + np.asarray(bh, f32)
        return np.ascontiguousarray(
            np.stack([s[g * H + k * HC: g * H + (k + 1) * HC] for g in range(4)],
                     axis=1))                                        # [128, 4]

    def pack_hT(h_l):   # [B, H] -> [128, KT*B]
        hT = np.ascontiguousarray(h_l.T)                             # [H, B]
        return np.ascontiguousarray(
            hT.reshape(KT, 128, B).transpose(1, 0, 2).reshape(128, KT * B))

    if MM_DTYPE == "bfloat16":
        import ml_dtypes
        mnp = ml_dtypes.bfloat16
    else:
        mnp = np.float32

    def mcast(a):
        return np.ascontiguousarray(a.astype(mnp))

    eye = mcast(np.eye(128, dtype=f32))
    in_maps = []
    for k in range(NCORES):
        m = {"eye": eye}
        m["whh0"] = mcast(pack_w(W_hh0, k))
        m["wih1"] = mcast(pack_w(W_ih1, k))
        m["whh1"] = mcast(pack_w(W_hh1, k))
        m["wih2"] = mcast(pack_w(W_ih2, k))
        m["whh2"] = mcast(pack_w(W_hh2, k))
        m["b1"] = pack_bias(b_ih1, b_hh1, k)
        m["b2"] = pack_bias(b_ih2, b_hh2, k)
        gx = np.stack(
            [gx0_full[:, g * H + k * HC: g * H + (k + 1) * HC].T for g in range(4)],
            axis=1)                                                  # [128, 4, B]
        m["gx0"] = mcast(gx.reshape(128, 4 * B))
        m["fcw"] = mcast(fcw_pad[k * 128:(k + 1) * 128, :])          # [128, 208]
        for l in range(NLAYERS):
            m[f"h{l}i"] = mcast(pack_hT(h0_all[:, l * H:(l + 1) * H]))
            m[f"c{l}i"] = np.ascontiguousarray(
                c0_all[:, l * H:(l + 1) * H].T[k * HC:(k + 1) * HC, :])
        in_maps.append(m)
    return in_maps, fcb_pad


def _run(steps, in_maps, trace=False):
    from concourse import bass_utils
    nc = _get_nc(steps)
    return bass_utils.run_bass_kernel_spmd(
        nc, in_maps, core_ids=list(range(NCORES)), trace=trace)


def _assemble(results, steps, fcb_pad):
    # per-core K-shard partials [steps, FCP, B]: sum + bias -> [B, steps, 68, 3]
    full = results[0]["out"].astype(np.float64)
    for k in range(1, NCORES):
        full += results[k]["out"]
    full = full.astype(np.float32) + fcb_pad[None, :, None]
    full = full.transpose(2, 0, 1)[:, :, :OUT]       # [B, steps, 204]
    return np.ascontiguousarray(full.reshape(B, steps, 68, 3).astype(np.float32))


def kernel(**inputs) -> np.ndarray:
    in_maps, fcb_pad = _prep_inputs(**inputs)
    last_err = None
    for attempt in range(3):
        try:
            res = _run(STEPS, in_maps, trace=False)
            return _assemble(res.results, STEPS, fcb_pad)
        except Exception as e:  # transient NRT device-unrecoverable errors
            last_err = e
    raise last_err



# revision 4
# speedup vs baseline: 1.0274x; 1.0274x over previous
"""DecoderRNN (3-layer LSTM, H=1024, B=256, L=128 steps) on 8 trn2 NeuronCores.

Strategy: 8-way tensor parallel over hidden units (feature-major layout).
Core k owns hidden units [128k, 128k+128) of every layer, holding the matching
512 gate columns of each weight matrix SBUF-resident for all 128 timesteps.
Activations live transposed (h.T : [hidden, batch]); after each layer-step the
128-row h.T chunk is AllGather'd so every core has the full h.T for the next
matmul. Layers are wavefront-pipelined (layer l does step t at phase t+l) so
each phase depends only on the previous phase's three independent AllGathers.
The time-invariant layer-0 input projection (x_in @ W_ih0 + biases) is
precomputed on the host and injected into PSUM with an identity matmul.
"""
import sys
import os

sys.path.insert(0, "/opt/trn_rl_repo")

import numpy as np

B = 256          # batch
H = 1024         # hidden size
NLAYERS = 3
STEPS = 128      # decoded sequence length
OUT = 204        # output size (68*3)
NUM_CLASSES = 12
IN0 = OUT + NUM_CLASSES
NCORES = 8
HC = H // NCORES          # hidden chunk per core = 128
GC = 4 * HC               # gate columns per core = 512
KT = H // 128             # contraction k-tiles = 8
FCC = 26                  # legacy fc shard width (unused in K-sharded fc)
FCP = 208                 # padded fc output size (>= 204)

MM_DTYPE = os.environ.get("KERNEL_MM_DTYPE", "bfloat16")  # bfloat16|float32r|float32

_BUILD_CACHE = {}


def _build(steps: int):
    """Build + compile the SPMD Bass program (same NEFF for all 8 cores)."""
    import concourse.bass as bass
    import concourse.bacc as bacc
    import concourse.tile as tile
    import concourse.mybir as mybir

    f32 = mybir.dt.float32
    mdt = getattr(mybir.dt, MM_DTYPE)   # dtype of all matmul operands
    AF = mybir.ActivationFunctionType

    nc = bacc.Bacc("TRN2", target_bir_lowering=False, debug=False,
                   num_devices=NCORES)

    # ---- kernel I/O -----------------------------------------------------
    w_names = ["whh0", "wih1", "whh1", "wih2", "whh2"]
    w_ext = {n: nc.dram_tensor(n, [128, KT * GC], mdt, kind="ExternalInput")
             for n in w_names}
    gx0_ext = nc.dram_tensor("gx0", [128, 4 * B], mdt, kind="ExternalInput")
    b_ext = {l: nc.dram_tensor(f"b{l}", [128, 4], f32, kind="ExternalInput")
             for l in (1, 2)}
    fcw_ext = nc.dram_tensor("fcw", [128, FCP], mdt, kind="ExternalInput")
    eye_ext = nc.dram_tensor("eye", [128, 128], mdt, kind="ExternalInput")
    hinit_ext = [nc.dram_tensor(f"h{l}i", [128, KT * B], mdt, kind="ExternalInput")
                 for l in range(NLAYERS)]
    cinit_ext = [nc.dram_tensor(f"c{l}i", [HC, B], f32, kind="ExternalInput")
                 for l in range(NLAYERS)]
    out_ext = nc.dram_tensor("out", [steps, FCP, B], f32, kind="ExternalOutput")

    rg = [list(range(NCORES))]

    with tile.TileContext(nc) as tc:
        with tc.tile_pool(name="const", bufs=1) as cpool, \
             tc.tile_pool(name="work", bufs=2) as wpool, \
             tc.tile_pool(name="psum", bufs=2, space="PSUM") as ppool, \
             tc.tile_pool(name="dram", bufs=2, space="DRAM") as dpool:

            # ---- load constants into SBUF (resident for whole kernel) ----
            w_sb = {}
            for n in w_names:
                t = cpool.tile([128, KT * GC], mdt, name=f"sb_{n}")
                nc.sync.dma_start(t[:], w_ext[n][:])
                w_sb[n] = t
            gx0 = cpool.tile([128, 4 * B], mdt, name="sb_gx0")
            nc.sync.dma_start(gx0[:], gx0_ext[:])
            b_sb = {}
            for l in (1, 2):
                t = cpool.tile([128, 4], f32, name=f"sb_b{l}")
                nc.sync.dma_start(t[:], b_ext[l][:])
                b_sb[l] = t
            fcw = cpool.tile([128, FCP], mdt, name="sb_fcw")
            nc.sync.dma_start(fcw[:], fcw_ext[:])
            eye = cpool.tile([128, 128], mdt, name="sb_eye")
            nc.sync.dma_start(eye[:], eye_ext[:])

            h_buf = []
            c_buf = []
            for l in range(NLAYERS):
                ht = wpool.tile([128, KT * B], mdt, tag=f"h{l}", bufs=3,
                                name=f"h{l}_init")
                nc.sync.dma_start(ht[:], hinit_ext[l][:])
                h_buf.append(ht)
                ct = wpool.tile([HC, B], f32, tag=f"c{l}", name=f"c{l}_init")
                nc.sync.dma_start(ct[:], cinit_ext[l][:])
                c_buf.append(ct)

            w_hh = {0: w_sb["whh0"], 1: w_sb["whh1"], 2: w_sb["whh2"]}
            w_ih = {1: w_sb["wih1"], 2: w_sb["wih2"]}

            def emit_layer(l, p):
                """LSTM layer l consuming h_buf/c_buf state; returns AG out."""
                h_self = h_buf[l]
                h_below = h_buf[l - 1] if l > 0 else None
                # gate order in weights: i,f,g,o ; emit f,i,g,o so the cell
                # can start as early as possible.
                acts = {}
                for g in (1, 0, 2, 3):
                    ps = ppool.tile([HC, B], f32, tag="gates", bufs=6,
                                    name=f"ps_l{l}_p{p}_g{g}")
                    n_mm = (1 if l == 0 else 0) + KT * (1 if l == 0 else 2)
                    idx = 0
                    if l == 0:
                        nc.tensor.matmul(ps[:], eye[:],
                                         gx0[:, g * B:(g + 1) * B],
                                         start=True, stop=(idx == n_mm - 1))
                        idx += 1
                    for kt in range(KT):
                        nc.tensor.matmul(
                            ps[:],
                            w_hh[l][:, (kt * 4 + g) * HC:(kt * 4 + g + 1) * HC],
                            h_self[:, kt * B:(kt + 1) * B],
                            start=(idx == 0), stop=(idx == n_mm - 1))
                        idx += 1
                    if l > 0:
                        for kt in range(KT):
                            nc.tensor.matmul(
                                ps[:],
                                w_ih[l][:, (kt * 4 + g) * HC:(kt * 4 + g + 1) * HC],
                                h_below[:, kt * B:(kt + 1) * B],
                                start=False, stop=(idx == n_mm - 1))
                            idx += 1
                    a = wpool.tile([HC, B], f32, tag="gact", bufs=10,
                                   name=f"act_l{l}_p{p}_g{g}")
                    func = AF.Tanh if g == 2 else AF.Sigmoid
                    bias = b_sb[l][:, g:g + 1] if l > 0 else 0.0
                    nc.scalar.activation(a[:], ps[:], func, bias=bias)
                    acts[g] = a

                t_fc = wpool.tile([HC, B], f32, tag="tfc", bufs=3, name=f"tfc{l}_{p}")
                nc.vector.tensor_mul(t_fc[:], acts[1][:], c_buf[l][:])
                t_ig = wpool.tile([HC, B], f32, tag="tig", bufs=3, name=f"tig{l}_{p}")
                nc.vector.tensor_mul(t_ig[:], acts[0][:], acts[2][:])
                c_new = wpool.tile([HC, B], f32, tag=f"c{l}", name=f"c{l}_p{p}")
                nc.vector.tensor_add(c_new[:], t_fc[:], t_ig[:])
                c_buf[l] = c_new
                th = wpool.tile([HC, B], f32, tag="th", bufs=3, name=f"th{l}_{p}")
                nc.scalar.activation(th[:], c_new[:], AF.Tanh)
                hch = wpool.tile([HC, B], mdt, tag="hch", bufs=6, name=f"hch{l}_{p}")
                nc.vector.tensor_mul(hch[:], acts[3][:], th[:])
                return hch

            def emit_fc(t, hch2):
                # K-sharded fc: partial = fc_W[128k:128k+128, :].T @ h2_chunk
                # (local, no gathered h2 needed); host sums the 8 partials.
                ps1 = ppool.tile([128, B], f32, tag="fc1", bufs=1,
                                 name=f"psfc1_{t}")
                nc.tensor.matmul(ps1[:], fcw[:, 0:128], hch2[:],
                                 start=True, stop=True)
                ps2 = ppool.tile([FCP - 128, B], f32, tag="fc2", bufs=1,
                                 name=f"psfc2_{t}")
                nc.tensor.matmul(ps2[:], fcw[:, 128:FCP], hch2[:],
                                 start=True, stop=True)
                o1 = wpool.tile([128, B], f32, tag="osb1", bufs=2, name=f"o1_{t}")
                nc.vector.tensor_copy(o1[:], ps1[:])
                o2 = wpool.tile([FCP - 128, B], f32, tag="osb2", bufs=2,
                                name=f"o2_{t}")
                nc.vector.tensor_copy(o2[:], ps2[:])
                # out stores go on the gpsimd SWDGE queue: the sync queue is
                # reserved for AG-landing DMAs (the phase-critical path).
                nc.gpsimd.dma_start(out_ext[t, 0:128, :], o1[:])
                nc.gpsimd.dma_start(out_ext[t, 128:FCP, :], o2[:])

            def emit_ag(layers_hch, p):
                """AllGather the given layers' chunks (merged when >1)."""
                nl = len(layers_hch)
                tag = "ag" + "".join(str(l) for l in layers_hch)
                agi = dpool.tile([nl * HC, B], mdt, tag=f"i{tag}",
                                 name=f"agi{tag}_p{p}")
                for i, (l, hch) in enumerate(layers_hch.items()):
                    # staging on the scalar HWDGE queue: fast descriptor gen,
                    # and it lands right after this layer's activations there;
                    # sync stays landing-only, gpsimd carries trigger + outs
                    nc.scalar.dma_start(agi[i * HC:(i + 1) * HC, :], hch[:])
                ago = dpool.tile([nl * H, B], mdt, tag=f"o{tag}",
                                 addr_space="Shared", name=f"ago{tag}_p{p}")
                nc.gpsimd.collective_compute(
                    "AllGather", mybir.AluOpType.bypass, replica_groups=rg,
                    ins=[agi[:].opt()], outs=[ago[:].opt()])
                # land gathers into fresh SBUF h tiles on the sync HWDGE
                # queue, which carries ONLY landings: the wait-for-AG then
                # never head-of-line blocks activations (scalar) or bounce
                # DMAs (gpsimd), and the landing issues the moment the AG
                # completes — it is the phase-critical chain.
                ago_v = ago.rearrange("(j l p) f -> l p j f", l=nl, p=HC)
                for i, l in enumerate(layers_hch):
                    h_new = wpool.tile([128, KT * B], mdt, tag=f"h{l}", bufs=3,
                                       name=f"h{l}_p{p}")
                    nc.sync.dma_start(
                        h_new.rearrange("p (j f) -> p j f", j=KT), ago_v[i])
                    h_buf[l] = h_new

            # ---- wavefront over phases -----------------------------------
            for p in range(steps + NLAYERS - 1):
                hchs = {}
                if p <= steps - 1:
                    hchs[0] = emit_layer(0, p)
                if 1 <= p <= steps:
                    hchs[1] = emit_layer(1, p)
                if 2 <= p <= steps + 1:
                    hchs[2] = emit_layer(2, p)
                    emit_fc(p - 2, hchs[2])
                # solo per-layer AGs: each fires right after its cell so
                # next-phase consumers unblock as early as possible
                for l, hch in hchs.items():
                    emit_ag({l: hch}, p)

    nc.compile()
    return nc


def _get_nc(steps: int):
    key = (steps, MM_DTYPE)
    if key not in _BUILD_CACHE:
        _BUILD_CACHE[key] = _build(steps)
    return _BUILD_CACHE[key]


def _prep_inputs(inputs, W_ih0, W_hh0, b_ih0, b_hh0, W_ih1, W_hh1, b_ih1, b_hh1,
                 W_ih2, W_hh2, b_ih2, b_hh2, fc_W, fc_b, inh_W, inh_b,
                 inc_W, inc_b, labels):
    """Host-side shard prep: all arrays already in the on-device layout."""
    f32 = np.float32
    frame = np.asarray(inputs, f32).reshape(B, OUT)
    onehot = np.zeros((B, NUM_CLASSES), f32)
    onehot[:, int(labels)] = 1.0
    x_in = np.concatenate([frame, onehot], axis=1)                  # [B, 216]

    h0_all = frame @ np.asarray(inh_W, f32) + np.asarray(inh_b, f32)  # [B, 3H]
    c0_all = frame @ np.asarray(inc_W, f32) + np.asarray(inc_b, f32)
    gx0_full = (x_in @ np.asarray(W_ih0, f32)
                + np.asarray(b_ih0, f32) + np.asarray(b_hh0, f32))  # [B, 4H]

    fcw_pad = np.zeros((H, FCP), f32)
    fcw_pad[:, :OUT] = np.asarray(fc_W, f32)
    fcb_pad = np.zeros(FCP, f32)
    fcb_pad[:OUT] = np.asarray(fc_b, f32)

    def pack_w(Wfull, k):
        Wfull = np.asarray(Wfull, f32)
        cols = np.concatenate(
            [Wfull[:, g * H + k * HC: g * H + (k + 1) * HC] for g in range(4)],
            axis=1)                                                  # [K, 512]
        kt = Wfull.shape[0] // 128
        return np.ascontiguousarray(
            cols.reshape(kt, 128, 4, HC).transpose(1, 0, 2, 3).reshape(128, kt * GC))

    def pack_bias(bi, bh, k):
        s = np.asarray(bi, f32) + np.asarray(bh, f32)
        return np.ascontiguousarray(
            np.stack([s[g * H + k * HC: g * H + (k + 1) * HC] for g in range(4)],
                     axis=1))                                        # [128, 4]

    def pack_hT(h_l):   # [B, H] -> [128, KT*B]
        hT = np.ascontiguousarray(h_l.T)                             # [H, B]
        return np.ascontiguousarray(
            hT.reshape(KT, 128, B).transpose(1, 0, 2).reshape(128, KT * B))

    if MM_DTYPE == "bfloat16":
        import ml_dtypes
        mnp = ml_dtypes.bfloat16
    else:
        mnp = np.float32

    def mcast(a):
        return np.ascontiguousarray(a.astype(mnp))

    eye = mcast(np.eye(128, dtype=f32))
    in_maps = []
    for k in range(NCORES):
        m = {"eye": eye}
        m["whh0"] = mcast(pack_w(W_hh0, k))
        m["wih1"] = mcast(pack_w(W_ih1, k))
        m["whh1"] = mcast(pack_w(W_hh1, k))
        m["wih2"] = mcast(pack_w(W_ih2, k))
        m["whh2"] = mcast(pack_w(W_hh2, k))
        m["b1"] = pack_bias(b_ih1, b_hh1, k)
        m["b2"] = pack_bias(b_ih2, b_hh2, k)
        gx = np.stack(
            [gx0_full[:, g * H + k * HC: g * H + (k + 1) * HC].T for g in range(4)],
            axis=1)                                                  # [128, 4, B]
        m["gx0"] = mcast(gx.reshape(128, 4 * B))
        m["fcw"] = mcast(fcw_pad[k * 128:(k + 1) * 128, :])          # [128, 208]
        for l in range(NLAYERS):
            m[f"h{l}i"] = mcast(pack_hT(h0_all[:, l * H:(l + 1) * H]))
            m[f"c{l}i"] = np.ascontiguousarray(
                c0_all[:, l * H:(l + 1) * H].T[k * HC:(k + 1) * HC, :])
        in_maps.append(m)
    return in_maps, fcb_pad


def _run(steps, in_maps, trace=False):
    from concourse import bass_utils
    nc = _get_nc(steps)
    return bass_utils.run_bass_kernel_spmd(
        nc, in_maps, core_ids=list(range(NCORES)), trace=trace)


def _assemble(results, steps, fcb_pad):
    # per-core K-shard partials [steps, FCP, B]: sum + bias -> [B, steps, 68, 3]
    full = results[0]["out"].astype(np.float64)
    for k in range(1, NCORES):
        full += results[k]["out"]
    full = full.astype(np.float32) + fcb_pad[None, :, None]
    full = full.transpose(2, 0, 1)[:, :, :OUT]       # [B, steps, 204]
    return np.ascontiguousarray(full.reshape(B, steps, 68, 3).astype(np.float32))


def kernel(**inputs) -> np.ndarray:
    in_maps, fcb_pad = _prep_inputs(**inputs)
    last_err = None
    for attempt in range(3):
        try:
            res = _run(STEPS, in_maps, trace=False)
            return _assemble(res.results, STEPS, fcb_pad)
        except Exception as e:  # transient NRT device-unrecoverable errors
            last_err = e
    raise last_err



# revision 6
# speedup vs baseline: 3808.1789x; 3706.6346x over previous
"""DecoderRNN (3-layer LSTM, H=1024, B=256, L=128 steps) on 8 trn2 NeuronCores.

Strategy: 8-way tensor parallel over hidden units (feature-major layout).
Core k owns hidden units [128k, 128k+128) of every layer, holding the matching
512 gate columns of each weight matrix SBUF-resident for all 128 timesteps.
Activations live transposed (h.T : [hidden, batch]); after each layer-step the
128-row h.T chunk is AllGather'd so every core has the full h.T for the next
matmul. Layers are wavefront-pipelined (layer l does step t at phase t+l) so
each phase depends only on the previous phase's three independent AllGathers.
The time-invariant layer-0 input projection (x_in @ W_ih0 + biases) is
precomputed on the host and injected into PSUM with an identity matmul.

DMA queue layout (load-bearing for perf — see trace analysis 2026-08-08):
AG-landing DMAs ride alone on the sync HWDGE queue (issue the moment the
collective completes; they are the phase-critical chain), AG staging rides
the scalar HWDGE queue behind that layer's activations, and AG triggers +
output stores ride gpsimd.  Measured 4.57 ms HW vs 6.56 ms for the original
(landings behind activations on scalar left the PE idling 4-19 us per layer
per phase, which also HAM-throttled the PE clock to 1.2 GHz).
"""
import sys
import os

sys.path.insert(0, "/opt/trn_rl_repo")

import numpy as np

B = 256          # batch
H = 1024         # hidden size
NLAYERS = 3
STEPS = 128      # decoded sequence length
OUT = 204        # output size (68*3)
NUM_CLASSES = 12
IN0 = OUT + NUM_CLASSES
NCORES = 8
HC = H // NCORES          # hidden chunk per core = 128
GC = 4 * HC               # gate columns per core = 512
KT = H // 128             # contraction k-tiles = 8
FCC = 26                  # legacy fc shard width (unused in K-sharded fc)
FCP = 208                 # padded fc output size (>= 204)

MM_DTYPE = os.environ.get("KERNEL_MM_DTYPE", "bfloat16")  # bfloat16|float32r|float32

_BUILD_CACHE = {}


def _build(steps: int):
    """Build + compile the SPMD Bass program (same NEFF for all 8 cores)."""
    import concourse.bass as bass
    import concourse.bacc as bacc
    import concourse.tile as tile
    import concourse.mybir as mybir

    f32 = mybir.dt.float32
    mdt = getattr(mybir.dt, MM_DTYPE)   # dtype of all matmul operands
    AF = mybir.ActivationFunctionType

    nc = bacc.Bacc("TRN2", target_bir_lowering=False, debug=False,
                   num_devices=NCORES)

    # ---- kernel I/O -----------------------------------------------------
    w_names = ["whh0", "wih1", "whh1", "wih2", "whh2"]
    w_ext = {n: nc.dram_tensor(n, [128, KT * GC], mdt, kind="ExternalInput")
             for n in w_names}
    gx0_ext = nc.dram_tensor("gx0", [128, 4 * B], mdt, kind="ExternalInput")
    b_ext = {l: nc.dram_tensor(f"b{l}", [128, 4], f32, kind="ExternalInput")
             for l in (1, 2)}
    fcw_ext = nc.dram_tensor("fcw", [128, FCP], mdt, kind="ExternalInput")
    eye_ext = nc.dram_tensor("eye", [128, 128], mdt, kind="ExternalInput")
    hinit_ext = [nc.dram_tensor(f"h{l}i", [128, KT * B], mdt, kind="ExternalInput")
                 for l in range(NLAYERS)]
    cinit_ext = [nc.dram_tensor(f"c{l}i", [HC, B], f32, kind="ExternalInput")
                 for l in range(NLAYERS)]
    out_ext = nc.dram_tensor("out", [steps, FCP, B], f32, kind="ExternalOutput")

    rg = [list(range(NCORES))]

    with tile.TileContext(nc) as tc:
        with tc.tile_pool(name="const", bufs=1) as cpool, \
             tc.tile_pool(name="work", bufs=2) as wpool, \
             tc.tile_pool(name="psum", bufs=2, space="PSUM") as ppool, \
             tc.tile_pool(name="dram", bufs=2, space="DRAM") as dpool:

            # ---- load constants into SBUF (resident for whole kernel) ----
            w_sb = {}
            for n in w_names:
                t = cpool.tile([128, KT * GC], mdt, name=f"sb_{n}")
                nc.sync.dma_start(t[:], w_ext[n][:])
                w_sb[n] = t
            gx0 = cpool.tile([128, 4 * B], mdt, name="sb_gx0")
            nc.sync.dma_start(gx0[:], gx0_ext[:])
            b_sb = {}
            for l in (1, 2):
                t = cpool.tile([128, 4], f32, name=f"sb_b{l}")
                nc.sync.dma_start(t[:], b_ext[l][:])
                b_sb[l] = t
            fcw = cpool.tile([128, FCP], mdt, name="sb_fcw")
            nc.sync.dma_start(fcw[:], fcw_ext[:])
            eye = cpool.tile([128, 128], mdt, name="sb_eye")
            nc.sync.dma_start(eye[:], eye_ext[:])

            h_buf = []
            c_buf = []
            for l in range(NLAYERS):
                ht = wpool.tile([128, KT * B], mdt, tag=f"h{l}", bufs=3,
                                name=f"h{l}_init")
                nc.sync.dma_start(ht[:], hinit_ext[l][:])
                h_buf.append(ht)
                ct = wpool.tile([HC, B], f32, tag=f"c{l}", name=f"c{l}_init")
                nc.sync.dma_start(ct[:], cinit_ext[l][:])
                c_buf.append(ct)

            w_hh = {0: w_sb["whh0"], 1: w_sb["whh1"], 2: w_sb["whh2"]}
            w_ih = {1: w_sb["wih1"], 2: w_sb["wih2"]}

            def emit_layer(l, p):
                """LSTM layer l consuming h_buf/c_buf state; returns AG out."""
                h_self = h_buf[l]
                h_below = h_buf[l - 1] if l > 0 else None
                # gate order in weights: i,f,g,o ; emit f,i,g,o so the cell
                # can start as early as possible.
                acts = {}
                for g in (1, 0, 2, 3):
                    ps = ppool.tile([HC, B], f32, tag="gates", bufs=6,
                                    name=f"ps_l{l}_p{p}_g{g}")
                    n_mm = (1 if l == 0 else 0) + KT * (1 if l == 0 else 2)
                    idx = 0
                    if l == 0:
                        nc.tensor.matmul(ps[:], eye[:],
                                         gx0[:, g * B:(g + 1) * B],
                                         start=True, stop=(idx == n_mm - 1))
                        idx += 1
                    for kt in range(KT):
                        nc.tensor.matmul(
                            ps[:],
                            w_hh[l][:, (kt * 4 + g) * HC:(kt * 4 + g + 1) * HC],
                            h_self[:, kt * B:(kt + 1) * B],
                            start=(idx == 0), stop=(idx == n_mm - 1))
                        idx += 1
                    if l > 0:
                        for kt in range(KT):
                            nc.tensor.matmul(
                                ps[:],
                                w_ih[l][:, (kt * 4 + g) * HC:(kt * 4 + g + 1) * HC],
                                h_below[:, kt * B:(kt + 1) * B],
                                start=False, stop=(idx == n_mm - 1))
                            idx += 1
                    a = wpool.tile([HC, B], f32, tag="gact", bufs=10,
                                   name=f"act_l{l}_p{p}_g{g}")
                    func = AF.Tanh if g == 2 else AF.Sigmoid
                    bias = b_sb[l][:, g:g + 1] if l > 0 else 0.0
                    nc.scalar.activation(a[:], ps[:], func, bias=bias)
                    acts[g] = a

                t_fc = wpool.tile([HC, B], f32, tag="tfc", bufs=3, name=f"tfc{l}_{p}")
                nc.vector.tensor_mul(t_fc[:], acts[1][:], c_buf[l][:])
                t_ig = wpool.tile([HC, B], f32, tag="tig", bufs=3, name=f"tig{l}_{p}")
                nc.vector.tensor_mul(t_ig[:], acts[0][:], acts[2][:])
                c_new = wpool.tile([HC, B], f32, tag=f"c{l}", name=f"c{l}_p{p}")
                nc.vector.tensor_add(c_new[:], t_fc[:], t_ig[:])
                c_buf[l] = c_new
                th = wpool.tile([HC, B], f32, tag="th", bufs=3, name=f"th{l}_{p}")
                nc.scalar.activation(th[:], c_new[:], AF.Tanh)
                hch = wpool.tile([HC, B], mdt, tag="hch", bufs=6, name=f"hch{l}_{p}")
                nc.vector.tensor_mul(hch[:], acts[3][:], th[:])
                return hch

            def emit_fc(t, hch2):
                # K-sharded fc: partial = fc_W[128k:128k+128, :].T @ h2_chunk
                # (local, no gathered h2 needed); host sums the 8 partials.
                ps1 = ppool.tile([128, B], f32, tag="fc1", bufs=1,
                                 name=f"psfc1_{t}")
                nc.tensor.matmul(ps1[:], fcw[:, 0:128], hch2[:],
                                 start=True, stop=True)
                ps2 = ppool.tile([FCP - 128, B], f32, tag="fc2", bufs=1,
                                 name=f"psfc2_{t}")
                nc.tensor.matmul(ps2[:], fcw[:, 128:FCP], hch2[:],
                                 start=True, stop=True)
                o1 = wpool.tile([128, B], f32, tag="osb1", bufs=2, name=f"o1_{t}")
                nc.vector.tensor_copy(o1[:], ps1[:])
                o2 = wpool.tile([FCP - 128, B], f32, tag="osb2", bufs=2,
                                name=f"o2_{t}")
                nc.vector.tensor_copy(o2[:], ps2[:])
                # out stores go on the gpsimd SWDGE queue: the sync queue is
                # reserved for AG-landing DMAs (the phase-critical path).
                nc.gpsimd.dma_start(out_ext[t, 0:128, :], o1[:])
                nc.gpsimd.dma_start(out_ext[t, 128:FCP, :], o2[:])

            def emit_ag(layers_hch, p):
                """AllGather the given layers' chunks (merged when >1)."""
                nl = len(layers_hch)
                tag = "ag" + "".join(str(l) for l in layers_hch)
                agi = dpool.tile([nl * HC, B], mdt, tag=f"i{tag}",
                                 name=f"agi{tag}_p{p}")
                for i, (l, hch) in enumerate(layers_hch.items()):
                    # staging on the scalar HWDGE queue: fast descriptor gen,
                    # and it lands right after this layer's activations there;
                    # sync stays landing-only, gpsimd carries trigger + outs
                    nc.scalar.dma_start(agi[i * HC:(i + 1) * HC, :], hch[:])
                ago = dpool.tile([nl * H, B], mdt, tag=f"o{tag}",
                                 addr_space="Shared", name=f"ago{tag}_p{p}")
                nc.gpsimd.collective_compute(
                    "AllGather", mybir.AluOpType.bypass, replica_groups=rg,
                    ins=[agi[:].opt()], outs=[ago[:].opt()])
                # land gathers into fresh SBUF h tiles on the sync HWDGE
                # queue, which carries ONLY landings: the wait-for-AG then
                # never head-of-line blocks activations (scalar) or bounce
                # DMAs (gpsimd), and the landing issues the moment the AG
                # completes — it is the phase-critical chain.
                ago_v = ago.rearrange("(j l p) f -> l p j f", l=nl, p=HC)
                for i, l in enumerate(layers_hch):
                    h_new = wpool.tile([128, KT * B], mdt, tag=f"h{l}", bufs=3,
                                       name=f"h{l}_p{p}")
                    nc.sync.dma_start(
                        h_new.rearrange("p (j f) -> p j f", j=KT), ago_v[i])
                    h_buf[l] = h_new

            # ---- wavefront over phases -----------------------------------
            for p in range(steps + NLAYERS - 1):
                hchs = {}
                if p <= steps - 1:
                    hchs[0] = emit_layer(0, p)
                if 1 <= p <= steps:
                    hchs[1] = emit_layer(1, p)
                if 2 <= p <= steps + 1:
                    hchs[2] = emit_layer(2, p)
                    emit_fc(p - 2, hchs[2])
                # l0+l1 merged into one AG, l2 solo: 2 collectives/phase
                # instead of 3.  At 3/phase the single CC stream runs ~68%
                # duty and AG actives double under SDMA contention (7-10us),
                # stalling the PE on landings; the merge halves CC duty.
                # h0 lands ~cell_l1+stage+AG+land which still beats the
                # next phase's l0 block; l2's AG has 2 blocks of slack.
                to_ag = dict(hchs)
                if 0 in to_ag and 1 in to_ag:
                    emit_ag({0: to_ag.pop(0), 1: to_ag.pop(1)}, p)
                for l, hch in to_ag.items():
                    emit_ag({l: hch}, p)

    nc.compile()
    return nc


def _get_nc(steps: int):
    key = (steps, MM_DTYPE)
    if key not in _BUILD_CACHE:
        _BUILD_CACHE[key] = _build(steps)
    return _BUILD_CACHE[key]


def _prep_inputs(inputs, W_ih0, W_hh0, b_ih0, b_hh0, W_ih1, W_hh1, b_ih1, b_hh1,
                 W_ih2, W_hh2, b_ih2, b_hh2, fc_W, fc_b, inh_W, inh_b,
                 inc_W, inc_b, labels):
    """Host-side shard prep: all arrays already in the on-device layout."""
    f32 = np.float32
    frame = np.asarray(inputs, f32).reshape(B, OUT)
    onehot = np.zeros((B, NUM_CLASSES), f32)
    onehot[:, int(labels)] = 1.0
    x_in = np.concatenate([frame, onehot], axis=1)                  # [B, 216]

    h0_all = frame @ np.asarray(inh_W, f32) + np.asarray(inh_b, f32)  # [B, 3H]
    c0_all = frame @ np.asarray(inc_W, f32) + np.asarray(inc_b, f32)
    gx0_full = (x_in @ np.asarray(W_ih0, f32)
                + np.asarray(b_ih0, f32) + np.asarray(b_hh0, f32))  # [B, 4H]

    fcw_pad = np.zeros((H, FCP), f32)
    fcw_pad[:, :OUT] = np.asarray(fc_W, f32)
    fcb_pad = np.zeros(FCP, f32)
    fcb_pad[:OUT] = np.asarray(fc_b, f32)

    def pack_w(Wfull, k):
        Wfull = np.asarray(Wfull, f32)
        cols = np.concatenate(
            [Wfull[:, g * H + k * HC: g * H + (k + 1) * HC] for g in range(4)],
            axis=1)                                                  # [K, 512]
        kt = Wfull.shape[0] // 128
        return np.ascontiguousarray(
            cols.reshape(kt, 128, 4, HC).transpose(1, 0, 2, 3).reshape(128, kt * GC))

    def pack_bias(bi, bh, k):
        s = np.asarray(bi, f32) + np.asarray(bh, f32)
        return np.ascontiguousarray(
            np.stack([s[g * H + k * HC: g * H + (k + 1) * HC] for g in range(4)],
                     axis=1))                                        # [128, 4]

    def pack_hT(h_l):   # [B, H] -> [128, KT*B]
        hT = np.ascontiguousarray(h_l.T)                             # [H, B]
        return np.ascontiguousarray(
            hT.reshape(KT, 128, B).transpose(1, 0, 2).reshape(128, KT * B))

    if MM_DTYPE == "bfloat16":
        import ml_dtypes
        mnp = ml_dtypes.bfloat16
    else:
        mnp = np.float32

    def mcast(a):
        return np.ascontiguousarray(a.astype(mnp))

    eye = mcast(np.eye(128, dtype=f32))
    in_maps = []
    for k in range(NCORES):
        m = {"eye": eye}
        m["whh0"] = mcast(pack_w(W_hh0, k))
        m["wih1"] = mcast(pack_w(W_ih1, k))
        m["whh1"] = mcast(pack_w(W_hh1, k))
        m["wih2"] = mcast(pack_w(W_ih2, k))
        m["whh2"] = mcast(pack_w(W_hh2, k))
        m["b1"] = pack_bias(b_ih1, b_hh1, k)
        m["b2"] = pack_bias(b_ih2, b_hh2, k)
        gx = np.stack(
            [gx0_full[:, g * H + k * HC: g * H + (k + 1) * HC].T for g in range(4)],
            axis=1)                                                  # [128, 4, B]
        m["gx0"] = mcast(gx.reshape(128, 4 * B))
        m["fcw"] = mcast(fcw_pad[k * 128:(k + 1) * 128, :])          # [128, 208]
        for l in range(NLAYERS):
            m[f"h{l}i"] = mcast(pack_hT(h0_all[:, l * H:(l + 1) * H]))
            m[f"c{l}i"] = np.ascontiguousarray(
                c0_all[:, l * H:(l + 1) * H].T[k * HC:(k + 1) * HC, :])
        in_maps.append(m)
    return in_maps, fcb_pad


def _run(steps, in_maps, trace=False):
    from concourse import bass_utils
    nc = _get_nc(steps)
    return bass_utils.run_bass_kernel_spmd(
        nc, in_maps, core_ids=list(range(NCORES)), trace=trace)


def _assemble(results, steps, fcb_pad):
    # per-core K-shard partials [steps, FCP, B]: sum + bias -> [B, steps, 68, 3]
    full = results[0]["out"].astype(np.float64)
    for k in range(1, NCORES):
        full += results[k]["out"]
    full = full.astype(np.float32) + fcb_pad[None, :, None]
    full = full.transpose(2, 0, 1)[:, :, :OUT]       # [B, steps, 204]
    return np.ascontiguousarray(full.reshape(B, steps, 68, 3).astype(np.float32))


def kernel(**inputs) -> np.ndarray:
    in_maps, fcb_pad = _prep_inputs(**inputs)
    last_err = None
    for attempt in range(3):
        try:
            res = _run(STEPS, in_maps, trace=False)
            return _assemble(res.results, STEPS, fcb_pad)
        except Exception as e:  # transient NRT device-unrecoverable errors
            last_err = e
    raise last_err



# revision 7
# speedup vs baseline: 4091.0505x; 1.0743x over previous
"""DecoderRNN (3-layer LSTM, H=1024, B=256, L=128 steps) on 8 trn2 NeuronCores.

Strategy: 8-way tensor parallel over hidden units (feature-major layout).
Core k owns hidden units [128k, 128k+128) of every layer, holding the matching
512 gate columns of each weight matrix SBUF-resident for all 128 timesteps.
Activations live transposed (h.T : [hidden, batch]); after each layer-step the
128-row h.T chunk is AllGather'd so every core has the full h.T for the next
matmul. Layers are wavefront-pipelined (layer l does step t at phase t+l) so
each phase depends only on the previous phase's three independent AllGathers.
The time-invariant layer-0 input projection (x_in @ W_ih0 + biases) is
precomputed on the host and injected into PSUM with an identity matmul.

DMA queue layout (load-bearing for perf — see trace analysis 2026-08-08):
AG-landing DMAs ride alone on the sync HWDGE queue (issue the moment the
collective completes; they are the phase-critical chain), AG staging rides
the scalar HWDGE queue behind that layer's activations, and AG triggers +
output stores ride gpsimd.  Measured 4.57 ms HW vs 6.56 ms for the original
(landings behind activations on scalar left the PE idling 4-19 us per layer
per phase, which also HAM-throttled the PE clock to 1.2 GHz).
"""
import sys
import os

sys.path.insert(0, "/opt/trn_rl_repo")

import numpy as np

B = 256          # batch
H = 1024         # hidden size
NLAYERS = 3
STEPS = 128      # decoded sequence length
OUT = 204        # output size (68*3)
NUM_CLASSES = 12
IN0 = OUT + NUM_CLASSES
NCORES = 8
HC = H // NCORES          # hidden chunk per core = 128
GC = 4 * HC               # gate columns per core = 512
KT = H // 128             # contraction k-tiles = 8
FCC = 26                  # legacy fc shard width (unused in K-sharded fc)
FCP = 208                 # padded fc output size (>= 204)

MM_DTYPE = os.environ.get("KERNEL_MM_DTYPE", "bfloat16")  # bfloat16|float32r|float32

_BUILD_CACHE = {}


def _build(steps: int):
    """Build + compile the SPMD Bass program (same NEFF for all 8 cores)."""
    import concourse.bass as bass
    import concourse.bacc as bacc
    import concourse.tile as tile
    import concourse.mybir as mybir

    f32 = mybir.dt.float32
    mdt = getattr(mybir.dt, MM_DTYPE)   # dtype of all matmul operands
    AF = mybir.ActivationFunctionType

    nc = bacc.Bacc("TRN2", target_bir_lowering=False, debug=False,
                   num_devices=NCORES)

    # ---- kernel I/O -----------------------------------------------------
    w_names = ["whh0", "wih1", "whh1", "wih2", "whh2"]
    w_ext = {n: nc.dram_tensor(n, [128, KT * GC], mdt, kind="ExternalInput")
             for n in w_names}
    gx0_ext = nc.dram_tensor("gx0", [128, 4 * B], mdt, kind="ExternalInput")
    b_ext = {l: nc.dram_tensor(f"b{l}", [128, 4], f32, kind="ExternalInput")
             for l in (1, 2)}
    fcw_ext = nc.dram_tensor("fcw", [128, FCP], mdt, kind="ExternalInput")
    eye_ext = nc.dram_tensor("eye", [128, 128], mdt, kind="ExternalInput")
    hinit_ext = [nc.dram_tensor(f"h{l}i", [128, KT * B], mdt, kind="ExternalInput")
                 for l in range(NLAYERS)]
    cinit_ext = [nc.dram_tensor(f"c{l}i", [HC, B], f32, kind="ExternalInput")
                 for l in range(NLAYERS)]
    out_ext = nc.dram_tensor("out", [steps, FCP, B], f32, kind="ExternalOutput")

    rg = [list(range(NCORES))]

    with tile.TileContext(nc) as tc:
        with tc.tile_pool(name="const", bufs=1) as cpool, \
             tc.tile_pool(name="work", bufs=2) as wpool, \
             tc.tile_pool(name="psum", bufs=2, space="PSUM") as ppool, \
             tc.tile_pool(name="dram", bufs=2, space="DRAM") as dpool:

            # ---- load constants into SBUF (resident for whole kernel) ----
            w_sb = {}
            for n in w_names:
                t = cpool.tile([128, KT * GC], mdt, name=f"sb_{n}")
                nc.sync.dma_start(t[:], w_ext[n][:])
                w_sb[n] = t
            gx0 = cpool.tile([128, 4 * B], mdt, name="sb_gx0")
            nc.sync.dma_start(gx0[:], gx0_ext[:])
            b_sb = {}
            for l in (1, 2):
                t = cpool.tile([128, 4], f32, name=f"sb_b{l}")
                nc.sync.dma_start(t[:], b_ext[l][:])
                b_sb[l] = t
            fcw = cpool.tile([128, FCP], mdt, name="sb_fcw")
            nc.sync.dma_start(fcw[:], fcw_ext[:])
            eye = cpool.tile([128, 128], mdt, name="sb_eye")
            nc.sync.dma_start(eye[:], eye_ext[:])

            h_buf = []
            c_buf = []
            for l in range(NLAYERS):
                ht = wpool.tile([128, KT * B], mdt, tag=f"h{l}", bufs=3,
                                name=f"h{l}_init")
                nc.sync.dma_start(ht[:], hinit_ext[l][:])
                h_buf.append(ht)
                ct = wpool.tile([HC, B], f32, tag=f"c{l}", name=f"c{l}_init")
                nc.sync.dma_start(ct[:], cinit_ext[l][:])
                c_buf.append(ct)

            w_hh = {0: w_sb["whh0"], 1: w_sb["whh1"], 2: w_sb["whh2"]}
            w_ih = {1: w_sb["wih1"], 2: w_sb["wih2"]}

            def emit_layer(l, p):
                """LSTM layer l consuming h_buf/c_buf state; returns AG out."""
                h_self = h_buf[l]
                h_below = h_buf[l - 1] if l > 0 else None
                # gate order in weights: i,f,g,o ; emit f,i,g,o so the cell
                # can start as early as possible.
                acts = {}
                for g in (1, 0, 2, 3):
                    ps = ppool.tile([HC, B], f32, tag="gates", bufs=6,
                                    name=f"ps_l{l}_p{p}_g{g}")
                    n_mm = (1 if l == 0 else 0) + KT * (1 if l == 0 else 2)
                    idx = 0
                    if l == 0:
                        nc.tensor.matmul(ps[:], eye[:],
                                         gx0[:, g * B:(g + 1) * B],
                                         start=True, stop=(idx == n_mm - 1))
                        idx += 1
                    for kt in range(KT):
                        nc.tensor.matmul(
                            ps[:],
                            w_hh[l][:, (kt * 4 + g) * HC:(kt * 4 + g + 1) * HC],
                            h_self[:, kt * B:(kt + 1) * B],
                            start=(idx == 0), stop=(idx == n_mm - 1))
                        idx += 1
                    if l > 0:
                        for kt in range(KT):
                            nc.tensor.matmul(
                                ps[:],
                                w_ih[l][:, (kt * 4 + g) * HC:(kt * 4 + g + 1) * HC],
                                h_below[:, kt * B:(kt + 1) * B],
                                start=False, stop=(idx == n_mm - 1))
                            idx += 1
                    a = wpool.tile([HC, B], f32, tag="gact", bufs=10,
                                   name=f"act_l{l}_p{p}_g{g}")
                    func = AF.Tanh if g == 2 else AF.Sigmoid
                    bias = b_sb[l][:, g:g + 1] if l > 0 else 0.0
                    nc.scalar.activation(a[:], ps[:], func, bias=bias)
                    acts[g] = a

                t_fc = wpool.tile([HC, B], f32, tag="tfc", bufs=3, name=f"tfc{l}_{p}")
                nc.vector.tensor_mul(t_fc[:], acts[1][:], c_buf[l][:])
                t_ig = wpool.tile([HC, B], f32, tag="tig", bufs=3, name=f"tig{l}_{p}")
                nc.vector.tensor_mul(t_ig[:], acts[0][:], acts[2][:])
                c_new = wpool.tile([HC, B], f32, tag=f"c{l}", name=f"c{l}_p{p}")
                nc.vector.tensor_add(c_new[:], t_fc[:], t_ig[:])
                c_buf[l] = c_new
                th = wpool.tile([HC, B], f32, tag="th", bufs=3, name=f"th{l}_{p}")
                nc.scalar.activation(th[:], c_new[:], AF.Tanh)
                hch = wpool.tile([HC, B], mdt, tag="hch", bufs=6, name=f"hch{l}_{p}")
                nc.vector.tensor_mul(hch[:], acts[3][:], th[:])
                return hch

            def emit_fc(t, hch2):
                # K-sharded fc: partial = fc_W[128k:128k+128, :].T @ h2_chunk
                # (local, no gathered h2 needed); host sums the 8 partials.
                ps1 = ppool.tile([128, B], f32, tag="fc1", bufs=1,
                                 name=f"psfc1_{t}")
                nc.tensor.matmul(ps1[:], fcw[:, 0:128], hch2[:],
                                 start=True, stop=True)
                ps2 = ppool.tile([FCP - 128, B], f32, tag="fc2", bufs=1,
                                 name=f"psfc2_{t}")
                nc.tensor.matmul(ps2[:], fcw[:, 128:FCP], hch2[:],
                                 start=True, stop=True)
                o1 = wpool.tile([128, B], f32, tag="osb1", bufs=2, name=f"o1_{t}")
                nc.vector.tensor_copy(o1[:], ps1[:])
                o2 = wpool.tile([FCP - 128, B], f32, tag="osb2", bufs=2,
                                name=f"o2_{t}")
                nc.vector.tensor_copy(o2[:], ps2[:])
                # out stores go on the gpsimd SWDGE queue: the sync queue is
                # reserved for AG-landing DMAs (the phase-critical path).
                nc.gpsimd.dma_start(out_ext[t, 0:128, :], o1[:])
                nc.gpsimd.dma_start(out_ext[t, 128:FCP, :], o2[:])

            def emit_ag(layers_hch, p):
                """AllGather the given layers' chunks (merged when >1)."""
                nl = len(layers_hch)
                tag = "ag" + "".join(str(l) for l in layers_hch)
                agi = dpool.tile([nl * HC, B], mdt, tag=f"i{tag}",
                                 name=f"agi{tag}_p{p}")
                for i, (l, hch) in enumerate(layers_hch.items()):
                    # staging on the scalar HWDGE queue: fast descriptor gen,
                    # and it lands right after this layer's activations there;
                    # sync stays landing-only, gpsimd carries trigger + outs
                    nc.scalar.dma_start(agi[i * HC:(i + 1) * HC, :], hch[:])
                ago = dpool.tile([nl * H, B], mdt, tag=f"o{tag}",
                                 addr_space="Shared", name=f"ago{tag}_p{p}")
                nc.gpsimd.collective_compute(
                    "AllGather", mybir.AluOpType.bypass, replica_groups=rg,
                    ins=[agi[:].opt()], outs=[ago[:].opt()])
                # land gathers into fresh SBUF h tiles on the sync HWDGE
                # queue, which carries ONLY landings: the wait-for-AG then
                # never head-of-line blocks activations (scalar) or bounce
                # DMAs (gpsimd), and the landing issues the moment the AG
                # completes — it is the phase-critical chain.
                ago_v = ago.rearrange("(j l p) f -> l p j f", l=nl, p=HC)
                for i, l in enumerate(layers_hch):
                    h_new = wpool.tile([128, KT * B], mdt, tag=f"h{l}", bufs=3,
                                       name=f"h{l}_p{p}")
                    nc.sync.dma_start(
                        h_new.rearrange("p (j f) -> p j f", j=KT), ago_v[i])
                    h_buf[l] = h_new

            # ---- wavefront over phases -----------------------------------
            for p in range(steps + NLAYERS - 1):
                hchs = {}
                if p <= steps - 1:
                    hchs[0] = emit_layer(0, p)
                if 1 <= p <= steps:
                    hchs[1] = emit_layer(1, p)
                if 2 <= p <= steps + 1:
                    hchs[2] = emit_layer(2, p)
                    emit_fc(p - 2, hchs[2])
                # solo per-layer AGs: each fires right after its cell so
                # next-phase consumers unblock as early as possible.
                # (Merging l0+l1 into one AG was tried 2026-08-08: the
                # 128KB/rank merged collective ran ~12us active vs ~2x5us
                # solo and the phase serialized behind cell_l1 — 4.91ms
                # vs 4.57ms.  Keep solo.)
                for l, hch in hchs.items():
                    emit_ag({l: hch}, p)

    nc.compile()
    return nc


def _get_nc(steps: int):
    key = (steps, MM_DTYPE)
    if key not in _BUILD_CACHE:
        _BUILD_CACHE[key] = _build(steps)
    return _BUILD_CACHE[key]


def _prep_inputs(inputs, W_ih0, W_hh0, b_ih0, b_hh0, W_ih1, W_hh1, b_ih1, b_hh1,
                 W_ih2, W_hh2, b_ih2, b_hh2, fc_W, fc_b, inh_W, inh_b,
                 inc_W, inc_b, labels):
    """Host-side shard prep: all arrays already in the on-device layout."""
    f32 = np.float32
    frame = np.asarray(inputs, f32).reshape(B, OUT)
    onehot = np.zeros((B, NUM_CLASSES), f32)
    onehot[:, int(labels)] = 1.0
    x_in = np.concatenate([frame, onehot], axis=1)                  # [B, 216]

    h0_all = frame @ np.asarray(inh_W, f32) + np.asarray(inh_b, f32)  # [B, 3H]
    c0_all = frame @ np.asarray(inc_W, f32) + np.asarray(inc_b, f32)
    gx0_full = (x_in @ np.asarray(W_ih0, f32)
                + np.asarray(b_ih0, f32) + np.asarray(b_hh0, f32))  # [B, 4H]

    fcw_pad = np.zeros((H, FCP), f32)
    fcw_pad[:, :OUT] = np.asarray(fc_W, f32)
    fcb_pad = np.zeros(FCP, f32)
    fcb_pad[:OUT] = np.asarray(fc_b, f32)

    def pack_w(Wfull, k):
        Wfull = np.asarray(Wfull, f32)
        cols = np.concatenate(
            [Wfull[:, g * H + k * HC: g * H + (k + 1) * HC] for g in range(4)],
            axis=1)                                                  # [K, 512]
        kt = Wfull.shape[0] // 128
        return np.ascontiguousarray(
            cols.reshape(kt, 128, 4, HC).transpose(1, 0, 2, 3).reshape(128, kt * GC))

    def pack_bias(bi, bh, k):
        s = np.asarray(bi, f32) + np.asarray(bh, f32)
        return np.ascontiguousarray(
            np.stack([s[g * H + k * HC: g * H + (k + 1) * HC] for g in range(4)],
                     axis=1))                                        # [128, 4]

    def pack_hT(h_l):   # [B, H] -> [128, KT*B]
        hT = np.ascontiguousarray(h_l.T)                             # [H, B]
        return np.ascontiguousarray(
            hT.reshape(KT, 128, B).transpose(1, 0, 2).reshape(128, KT * B))

    if MM_DTYPE == "bfloat16":
        import ml_dtypes
        mnp = ml_dtypes.bfloat16
    else:
        mnp = np.float32

    def mcast(a):
        return np.ascontiguousarray(a.astype(mnp))

    eye = mcast(np.eye(128, dtype=f32))
    in_maps = []
    for k in range(NCORES):
        m = {"eye": eye}
        m["whh0"] = mcast(pack_w(W_hh0, k))
        m["wih1"] = mcast(pack_w(W_ih1, k))
        m["whh1"] = mcast(pack_w(W_hh1, k))
        m["wih2"] = mcast(pack_w(W_ih2, k))
        m["whh2"] = mcast(pack_w(W_hh2, k))
        m["b1"] = pack_bias(b_ih1, b_hh1, k)
        m["b2"] = pack_bias(b_ih2, b_hh2, k)
        gx = np.stack(
            [gx0_full[:, g * H + k * HC: g * H + (k + 1) * HC].T for g in range(4)],
            axis=1)                                                  # [128, 4, B]
        m["gx0"] = mcast(gx.reshape(128, 4 * B))
        m["fcw"] = mcast(fcw_pad[k * 128:(k + 1) * 128, :])          # [128, 208]
        for l in range(NLAYERS):
            m[f"h{l}i"] = mcast(pack_hT(h0_all[:, l * H:(l + 1) * H]))
            m[f"c{l}i"] = np.ascontiguousarray(
                c0_all[:, l * H:(l + 1) * H].T[k * HC:(k + 1) * HC, :])
        in_maps.append(m)
    return in_maps, fcb_pad


def _run(steps, in_maps, trace=False):
    from concourse import bass_utils
    nc = _get_nc(steps)
    return bass_utils.run_bass_kernel_spmd(
        nc, in_maps, core_ids=list(range(NCORES)), trace=trace)


def _assemble(results, steps, fcb_pad):
    # per-core K-shard partials [steps, FCP, B]: sum + bias -> [B, steps, 68, 3]
    full = results[0]["out"].astype(np.float64)
    for k in range(1, NCORES):
        full += results[k]["out"]
    full = full.astype(np.float32) + fcb_pad[None, :, None]
    full = full.transpose(2, 0, 1)[:, :, :OUT]       # [B, steps, 204]
    return np.ascontiguousarray(full.reshape(B, steps, 68, 3).astype(np.float32))


def kernel(**inputs) -> np.ndarray:
    in_maps, fcb_pad = _prep_inputs(**inputs)
    last_err = None
    for attempt in range(3):
        try:
            res = _run(STEPS, in_maps, trace=False)
            return _assemble(res.results, STEPS, fcb_pad)
        except Exception as e:  # transient NRT device-unrecoverable errors
            last_err = e
    raise last_err



# revision 12
# speedup vs baseline: 4487.9780x; 1.0970x over previous
"""DecoderRNN (3-layer LSTM, H=1024, B=256, L=128 steps) on 8 trn2 NeuronCores.

Strategy: 8-way tensor parallel over hidden units (feature-major layout).
Core k owns hidden units [128k, 128k+128) of every layer, holding the matching
512 gate columns of each weight matrix SBUF-resident for all 128 timesteps.
Activations live transposed (h.T : [hidden, batch]); after each layer-step the
128-row h.T chunk is AllGather'd so every core has the full h.T for the next
matmul. Layers are wavefront-pipelined (layer l does step t at phase t+l) so
each phase depends only on the previous phase's three independent AllGathers.
The time-invariant layer-0 input projection (x_in @ W_ih0 + biases) is
precomputed on the host and injected into PSUM with an identity matmul.

DMA queue layout (load-bearing for perf — see trace analysis 2026-08-08):
AG-landing DMAs ride alone on the sync HWDGE queue (issue the moment the
collective completes; they are the phase-critical chain), AG staging rides
the scalar HWDGE queue behind that layer's activations, and AG triggers +
output stores ride gpsimd.  Measured 4.57 ms HW vs 6.56 ms for the original
(landings behind activations on scalar left the PE idling 4-19 us per layer
per phase, which also HAM-throttled the PE clock to 1.2 GHz).
"""
import sys
import os

sys.path.insert(0, "/opt/trn_rl_repo")

import numpy as np

B = 256          # batch
H = 1024         # hidden size
NLAYERS = 3
STEPS = 128      # decoded sequence length
OUT = 204        # output size (68*3)
NUM_CLASSES = 12
IN0 = OUT + NUM_CLASSES
NCORES = 8
HC = H // NCORES          # hidden chunk per core = 128
GC = 4 * HC               # gate columns per core = 512
KT = H // 128             # contraction k-tiles = 8
FCC = 26                  # legacy fc shard width (unused in K-sharded fc)
FCP = 208                 # padded fc output size (>= 204)

MM_DTYPE = os.environ.get("KERNEL_MM_DTYPE", "bfloat16")  # bfloat16|float32r|float32

_BUILD_CACHE = {}


def _build(steps: int):
    """Build + compile the SPMD Bass program (same NEFF for all 8 cores)."""
    import concourse.bass as bass
    import concourse.bacc as bacc
    import concourse.tile as tile
    import concourse.mybir as mybir

    f32 = mybir.dt.float32
    mdt = getattr(mybir.dt, MM_DTYPE)   # dtype of all matmul operands
    AF = mybir.ActivationFunctionType

    nc = bacc.Bacc("TRN2", target_bir_lowering=False, debug=False,
                   num_devices=NCORES)

    # ---- kernel I/O -----------------------------------------------------
    w_names = ["whh0", "wih1", "whh1", "wih2", "whh2"]
    w_ext = {n: nc.dram_tensor(n, [128, KT * GC], mdt, kind="ExternalInput")
             for n in w_names}
    gx0_ext = nc.dram_tensor("gx0", [128, 4 * B], mdt, kind="ExternalInput")
    b_ext = {l: nc.dram_tensor(f"b{l}", [128, 4], f32, kind="ExternalInput")
             for l in (1, 2)}
    fcw_ext = nc.dram_tensor("fcw", [128, FCP], mdt, kind="ExternalInput")
    eye_ext = nc.dram_tensor("eye", [128, 128], mdt, kind="ExternalInput")
    hinit_ext = [nc.dram_tensor(f"h{l}i", [128, KT * B], mdt, kind="ExternalInput")
                 for l in range(NLAYERS)]
    cinit_ext = [nc.dram_tensor(f"c{l}i", [HC, B], f32, kind="ExternalInput")
                 for l in range(NLAYERS)]
    out_ext = nc.dram_tensor("out", [steps, FCP, B], f32, kind="ExternalOutput")

    rg = [list(range(NCORES))]

    with tile.TileContext(nc) as tc:
        with tc.tile_pool(name="const", bufs=1) as cpool, \
             tc.tile_pool(name="work", bufs=2) as wpool, \
             tc.tile_pool(name="psum", bufs=2, space="PSUM") as ppool, \
             tc.tile_pool(name="dram", bufs=2, space="DRAM") as dpool:

            # ---- load constants into SBUF (resident for whole kernel) ----
            w_sb = {}
            for n in w_names:
                t = cpool.tile([128, KT * GC], mdt, name=f"sb_{n}")
                nc.sync.dma_start(t[:], w_ext[n][:])
                w_sb[n] = t
            gx0 = cpool.tile([128, 4 * B], mdt, name="sb_gx0")
            nc.sync.dma_start(gx0[:], gx0_ext[:])
            b_sb = {}
            for l in (1, 2):
                t = cpool.tile([128, 4], f32, name=f"sb_b{l}")
                nc.sync.dma_start(t[:], b_ext[l][:])
                b_sb[l] = t
            fcw = cpool.tile([128, FCP], mdt, name="sb_fcw")
            nc.sync.dma_start(fcw[:], fcw_ext[:])
            eye = cpool.tile([128, 128], mdt, name="sb_eye")
            nc.sync.dma_start(eye[:], eye_ext[:])

            h_buf = []
            c_buf = []
            for l in range(NLAYERS):
                ht = wpool.tile([128, KT * B], mdt, tag=f"h{l}", bufs=3,
                                name=f"h{l}_init")
                nc.sync.dma_start(ht[:], hinit_ext[l][:])
                h_buf.append(ht)
                ct = wpool.tile([HC, B], f32, tag=f"c{l}", name=f"c{l}_init")
                nc.sync.dma_start(ct[:], cinit_ext[l][:])
                c_buf.append(ct)

            w_hh = {0: w_sb["whh0"], 1: w_sb["whh1"], 2: w_sb["whh2"]}
            w_ih = {1: w_sb["wih1"], 2: w_sb["wih2"]}

            def emit_layer(l, p):
                """LSTM layer l consuming h_buf/c_buf state; returns AG out."""
                h_self = h_buf[l]
                h_below = h_buf[l - 1] if l > 0 else None
                # gate order in weights: i,f,g,o ; emit f,i,g,o so the cell
                # can start as early as possible.
                acts = {}

                def emit_gate(g):
                    ps = ppool.tile([HC, B], f32, tag="gates", bufs=6,
                                    name=f"ps_l{l}_p{p}_g{g}")
                    n_mm = (1 if l == 0 else 0) + KT * (1 if l == 0 else 2)
                    idx = 0
                    if l == 0:
                        nc.tensor.matmul(ps[:], eye[:],
                                         gx0[:, g * B:(g + 1) * B],
                                         start=True, stop=(idx == n_mm - 1))
                        idx += 1
                    for kt in range(KT):
                        nc.tensor.matmul(
                            ps[:],
                            w_hh[l][:, (kt * 4 + g) * HC:(kt * 4 + g + 1) * HC],
                            h_self[:, kt * B:(kt + 1) * B],
                            start=(idx == 0), stop=(idx == n_mm - 1))
                        idx += 1
                    if l > 0:
                        for kt in range(KT):
                            nc.tensor.matmul(
                                ps[:],
                                w_ih[l][:, (kt * 4 + g) * HC:(kt * 4 + g + 1) * HC],
                                h_below[:, kt * B:(kt + 1) * B],
                                start=False, stop=(idx == n_mm - 1))
                            idx += 1
                    a = wpool.tile([HC, B], f32, tag="gact", bufs=10,
                                   name=f"act_l{l}_p{p}_g{g}")
                    func = AF.Tanh if g == 2 else AF.Sigmoid
                    bias = b_sb[l][:, g:g + 1] if l > 0 else 0.0
                    nc.scalar.activation(a[:], ps[:], func, bias=bias)
                    acts[g] = a

                # f, i, g gates first; c2 + tanh(c2) emitted BEFORE the o
                # gate's activation so tanh(c2) precedes it in the ACT queue
                # (it is ready earlier) — hch then waits only on o's act.
                for g in (1, 0, 2):
                    emit_gate(g)
                t_fc = wpool.tile([HC, B], f32, tag="tfc", bufs=3, name=f"tfc{l}_{p}")
                nc.vector.tensor_mul(t_fc[:], acts[1][:], c_buf[l][:])
                t_ig = wpool.tile([HC, B], f32, tag="tig", bufs=3, name=f"tig{l}_{p}")
                nc.vector.tensor_mul(t_ig[:], acts[0][:], acts[2][:])
                c_new = wpool.tile([HC, B], f32, tag=f"c{l}", name=f"c{l}_p{p}")
                nc.vector.tensor_add(c_new[:], t_fc[:], t_ig[:])
                c_buf[l] = c_new
                th = wpool.tile([HC, B], f32, tag="th", bufs=3, name=f"th{l}_{p}")
                nc.scalar.activation(th[:], c_new[:], AF.Tanh)
                emit_gate(3)
                hch = wpool.tile([HC, B], mdt, tag="hch", bufs=6, name=f"hch{l}_{p}")
                nc.vector.tensor_mul(hch[:], acts[3][:], th[:])
                return hch

            def emit_fc(t, hch2):
                # K-sharded fc: partial = fc_W[128k:128k+128, :].T @ h2_chunk
                # (local, no gathered h2 needed); host sums the 8 partials.
                ps1 = ppool.tile([128, B], f32, tag="fc1", bufs=1,
                                 name=f"psfc1_{t}")
                nc.tensor.matmul(ps1[:], fcw[:, 0:128], hch2[:],
                                 start=True, stop=True)
                ps2 = ppool.tile([FCP - 128, B], f32, tag="fc2", bufs=1,
                                 name=f"psfc2_{t}")
                nc.tensor.matmul(ps2[:], fcw[:, 128:FCP], hch2[:],
                                 start=True, stop=True)
                o1 = wpool.tile([128, B], f32, tag="osb1", bufs=2, name=f"o1_{t}")
                nc.vector.tensor_copy(o1[:], ps1[:])
                o2 = wpool.tile([FCP - 128, B], f32, tag="osb2", bufs=2,
                                name=f"o2_{t}")
                nc.vector.tensor_copy(o2[:], ps2[:])
                # out stores go on the gpsimd SWDGE queue: the sync queue is
                # reserved for AG-landing DMAs (the phase-critical path).
                nc.gpsimd.dma_start(out_ext[t, 0:128, :], o1[:])
                nc.gpsimd.dma_start(out_ext[t, 128:FCP, :], o2[:])

            def emit_ag(layers_hch, p):
                """AllGather the given layers' chunks (merged when >1)."""
                nl = len(layers_hch)
                tag = "ag" + "".join(str(l) for l in layers_hch)
                agi = dpool.tile([nl * HC, B], mdt, tag=f"i{tag}",
                                 name=f"agi{tag}_p{p}")
                for i, (l, hch) in enumerate(layers_hch.items()):
                    # staging on the scalar HWDGE queue: fast descriptor gen,
                    # and it lands right after this layer's activations there;
                    # sync stays landing-only, gpsimd carries trigger + outs
                    nc.scalar.dma_start(agi[i * HC:(i + 1) * HC, :], hch[:])
                ago = dpool.tile([nl * H, B], mdt, tag=f"o{tag}",
                                 addr_space="Shared", name=f"ago{tag}_p{p}")
                nc.gpsimd.collective_compute(
                    "AllGather", mybir.AluOpType.bypass, replica_groups=rg,
                    ins=[agi[:].opt()], outs=[ago[:].opt()])
                # land gathers into fresh SBUF h tiles on the sync HWDGE
                # queue, which carries ONLY landings: the wait-for-AG then
                # never head-of-line blocks activations (scalar) or bounce
                # DMAs (gpsimd), and the landing issues the moment the AG
                # completes — it is the phase-critical chain.
                ago_v = ago.rearrange("(j l p) f -> l p j f", l=nl, p=HC)
                new_h = {}
                for i, l in enumerate(layers_hch):
                    h_new = wpool.tile([128, KT * B], mdt, tag=f"h{l}", bufs=3,
                                       name=f"h{l}_p{p}")
                    nc.sync.dma_start(
                        h_new.rearrange("p (j f) -> p j f", j=KT), ago_v[i])
                    new_h[l] = h_new
                # NOTE: caller installs new_h into h_buf only after ALL of
                # this phase's emit_layer calls — installing it here made
                # layer l+1 consume the CURRENT phase's gather (off-by-one
                # step, rel err 3.3e-2, and serialized l+1 behind the AG).
                return new_h

            # ---- wavefront over phases -----------------------------------
            # Emission order is load-bearing: each layer's AG staging +
            # doorbell is emitted IMMEDIATELY after its cell, and the fc
            # copies/out-stores come LAST.  Emitting fc before the AGs put
            # the out DMAs ahead of the doorbells on the gpsimd queue, whose
            # wait-for-fc-copy then delayed ALL the phase's AG triggers to
            # the end of the compute (~12us late, traced 2026-08-08) — that
            # ordering bug, not collective speed, was the 35us phase.
            # (Merging l0+l1 into one AG was also tried: 128KB/rank merged
            # collective ran ~12us active vs ~2x5us solo — keep solo AGs.)
            for p in range(steps + NLAYERS - 1):
                new_h = {}
                if p <= steps - 1:
                    new_h.update(emit_ag({0: emit_layer(0, p)}, p))
                if 1 <= p <= steps:
                    new_h.update(emit_ag({1: emit_layer(1, p)}, p))
                if 2 <= p <= steps + 1:
                    hch2 = emit_layer(2, p)
                    new_h.update(emit_ag({2: hch2}, p))
                    emit_fc(p - 2, hch2)
                for l, t in new_h.items():
                    h_buf[l] = t

    nc.compile()
    return nc


def _get_nc(steps: int):
    key = (steps, MM_DTYPE)
    if key not in _BUILD_CACHE:
        _BUILD_CACHE[key] = _build(steps)
    return _BUILD_CACHE[key]


def _prep_inputs(inputs, W_ih0, W_hh0, b_ih0, b_hh0, W_ih1, W_hh1, b_ih1, b_hh1,
                 W_ih2, W_hh2, b_ih2, b_hh2, fc_W, fc_b, inh_W, inh_b,
                 inc_W, inc_b, labels):
    """Host-side shard prep: all arrays already in the on-device layout."""
    f32 = np.float32
    frame = np.asarray(inputs, f32).reshape(B, OUT)
    onehot = np.zeros((B, NUM_CLASSES), f32)
    onehot[:, int(labels)] = 1.0
    x_in = np.concatenate([frame, onehot], axis=1)                  # [B, 216]

    h0_all = frame @ np.asarray(inh_W, f32) + np.asarray(inh_b, f32)  # [B, 3H]
    c0_all = frame @ np.asarray(inc_W, f32) + np.asarray(inc_b, f32)
    gx0_full = (x_in @ np.asarray(W_ih0, f32)
                + np.asarray(b_ih0, f32) + np.asarray(b_hh0, f32))  # [B, 4H]

    fcw_pad = np.zeros((H, FCP), f32)
    fcw_pad[:, :OUT] = np.asarray(fc_W, f32)
    fcb_pad = np.zeros(FCP, f32)
    fcb_pad[:OUT] = np.asarray(fc_b, f32)

    def pack_w(Wfull, k):
        Wfull = np.asarray(Wfull, f32)
        cols = np.concatenate(
            [Wfull[:, g * H + k * HC: g * H + (k + 1) * HC] for g in range(4)],
            axis=1)                                                  # [K, 512]
        kt = Wfull.shape[0] // 128
        return np.ascontiguousarray(
            cols.reshape(kt, 128, 4, HC).transpose(1, 0, 2, 3).reshape(128, kt * GC))

    def pack_bias(bi, bh, k):
        s = np.asarray(bi, f32) + np.asarray(bh, f32)
        return np.ascontiguousarray(
            np.stack([s[g * H + k * HC: g * H + (k + 1) * HC] for g in range(4)],
                     axis=1))                                        # [128, 4]

    def pack_hT(h_l):   # [B, H] -> [128, KT*B]
        hT = np.ascontiguousarray(h_l.T)                             # [H, B]
        return np.ascontiguousarray(
            hT.reshape(KT, 128, B).transpose(1, 0, 2).reshape(128, KT * B))

    if MM_DTYPE == "bfloat16":
        import ml_dtypes
        mnp = ml_dtypes.bfloat16
    else:
        mnp = np.float32

    def mcast(a):
        return np.ascontiguousarray(a.astype(mnp))

    eye = mcast(np.eye(128, dtype=f32))
    in_maps = []
    for k in range(NCORES):
        m = {"eye": eye}
        m["whh0"] = mcast(pack_w(W_hh0, k))
        m["wih1"] = mcast(pack_w(W_ih1, k))
        m["whh1"] = mcast(pack_w(W_hh1, k))
        m["wih2"] = mcast(pack_w(W_ih2, k))
        m["whh2"] = mcast(pack_w(W_hh2, k))
        m["b1"] = pack_bias(b_ih1, b_hh1, k)
        m["b2"] = pack_bias(b_ih2, b_hh2, k)
        gx = np.stack(
            [gx0_full[:, g * H + k * HC: g * H + (k + 1) * HC].T for g in range(4)],
            axis=1)                                                  # [128, 4, B]
        m["gx0"] = mcast(gx.reshape(128, 4 * B))
        m["fcw"] = mcast(fcw_pad[k * 128:(k + 1) * 128, :])          # [128, 208]
        for l in range(NLAYERS):
            m[f"h{l}i"] = mcast(pack_hT(h0_all[:, l * H:(l + 1) * H]))
            m[f"c{l}i"] = np.ascontiguousarray(
                c0_all[:, l * H:(l + 1) * H].T[k * HC:(k + 1) * HC, :])
        in_maps.append(m)
    return in_maps, fcb_pad


def _run(steps, in_maps, trace=False):
    from concourse import bass_utils
    nc = _get_nc(steps)
    return bass_utils.run_bass_kernel_spmd(
        nc, in_maps, core_ids=list(range(NCORES)), trace=trace)


def _assemble(results, steps, fcb_pad):
    # per-core K-shard partials [steps, FCP, B]: sum + bias -> [B, steps, 68, 3]
    full = results[0]["out"].astype(np.float64)
    for k in range(1, NCORES):
        full += results[k]["out"]
    full = full.astype(np.float32) + fcb_pad[None, :, None]
    full = full.transpose(2, 0, 1)[:, :, :OUT]       # [B, steps, 204]
    return np.ascontiguousarray(full.reshape(B, steps, 68, 3).astype(np.float32))


def kernel(**inputs) -> np.ndarray:
    in_maps, fcb_pad = _prep_inputs(**inputs)
    last_err = None
    for attempt in range(3):
        try:
            res = _run(STEPS, in_maps, trace=False)
            return _assemble(res.results, STEPS, fcb_pad)
        except Exception as e:  # transient NRT device-unrecoverable errors
            last_err = e
    raise last_err



# revision 13
# speedup vs baseline: 4537.1289x; 1.0110x over previous
"""DecoderRNN (3-layer LSTM, H=1024, B=256, L=128 steps) on 8 trn2 NeuronCores.

Strategy: 8-way tensor parallel over hidden units (feature-major layout).
Core k owns hidden units [128k, 128k+128) of every layer, holding the matching
512 gate columns of each weight matrix SBUF-resident for all 128 timesteps.
Activations live transposed (h.T : [hidden, batch]); after each layer-step the
128-row h.T chunk is AllGather'd so every core has the full h.T for the next
matmul. Layers are wavefront-pipelined (layer l does step t at phase t+l) so
each phase depends only on the previous phase's three independent AllGathers.
The time-invariant layer-0 input projection (x_in @ W_ih0 + biases) is
precomputed on the host and injected into PSUM with an identity matmul.

DMA queue layout (load-bearing for perf — see trace analysis 2026-08-08):
AG-landing DMAs ride alone on the sync HWDGE queue (issue the moment the
collective completes; they are the phase-critical chain), AG staging rides
the scalar HWDGE queue behind that layer's activations, and AG triggers +
output stores ride gpsimd.  Measured 4.57 ms HW vs 6.56 ms for the original
(landings behind activations on scalar left the PE idling 4-19 us per layer
per phase, which also HAM-throttled the PE clock to 1.2 GHz).
"""
import sys
import os

sys.path.insert(0, "/opt/trn_rl_repo")

import numpy as np

B = 256          # batch
H = 1024         # hidden size
NLAYERS = 3
STEPS = 128      # decoded sequence length
OUT = 204        # output size (68*3)
NUM_CLASSES = 12
IN0 = OUT + NUM_CLASSES
NCORES = 8
HC = H // NCORES          # hidden chunk per core = 128
GC = 4 * HC               # gate columns per core = 512
KT = H // 128             # contraction k-tiles = 8
FCC = 26                  # legacy fc shard width (unused in K-sharded fc)
FCP = 208                 # padded fc output size (>= 204)

MM_DTYPE = os.environ.get("KERNEL_MM_DTYPE", "bfloat16")  # bfloat16|float32r|float32

_BUILD_CACHE = {}


def _build(steps: int):
    """Build + compile the SPMD Bass program (same NEFF for all 8 cores)."""
    import concourse.bass as bass
    import concourse.bacc as bacc
    import concourse.tile as tile
    import concourse.mybir as mybir

    f32 = mybir.dt.float32
    mdt = getattr(mybir.dt, MM_DTYPE)   # dtype of all matmul operands
    AF = mybir.ActivationFunctionType

    nc = bacc.Bacc("TRN2", target_bir_lowering=False, debug=False,
                   num_devices=NCORES)

    # ---- kernel I/O -----------------------------------------------------
    w_names = ["whh0", "wih1", "whh1", "wih2", "whh2"]
    w_ext = {n: nc.dram_tensor(n, [128, KT * GC], mdt, kind="ExternalInput")
             for n in w_names}
    gx0_ext = nc.dram_tensor("gx0", [128, 4 * B], mdt, kind="ExternalInput")
    b_ext = {l: nc.dram_tensor(f"b{l}", [128, 4], f32, kind="ExternalInput")
             for l in (1, 2)}
    fcw_ext = nc.dram_tensor("fcw", [128, FCP], mdt, kind="ExternalInput")
    eye_ext = nc.dram_tensor("eye", [128, 128], mdt, kind="ExternalInput")
    hinit_ext = [nc.dram_tensor(f"h{l}i", [128, KT * B], mdt, kind="ExternalInput")
                 for l in range(NLAYERS)]
    cinit_ext = [nc.dram_tensor(f"c{l}i", [HC, B], f32, kind="ExternalInput")
                 for l in range(NLAYERS)]
    out_ext = nc.dram_tensor("out", [steps, FCP, B], mdt, kind="ExternalOutput")

    rg = [list(range(NCORES))]

    with tile.TileContext(nc) as tc:
        with tc.tile_pool(name="const", bufs=1) as cpool, \
             tc.tile_pool(name="work", bufs=2) as wpool, \
             tc.tile_pool(name="psum", bufs=2, space="PSUM") as ppool, \
             tc.tile_pool(name="dram", bufs=2, space="DRAM") as dpool:

            # ---- load constants into SBUF (resident for whole kernel) ----
            w_sb = {}
            for n in w_names:
                t = cpool.tile([128, KT * GC], mdt, name=f"sb_{n}")
                nc.sync.dma_start(t[:], w_ext[n][:])
                w_sb[n] = t
            gx0 = cpool.tile([128, 4 * B], mdt, name="sb_gx0")
            nc.sync.dma_start(gx0[:], gx0_ext[:])
            b_sb = {}
            for l in (1, 2):
                t = cpool.tile([128, 4], f32, name=f"sb_b{l}")
                nc.sync.dma_start(t[:], b_ext[l][:])
                b_sb[l] = t
            fcw = cpool.tile([128, FCP], mdt, name="sb_fcw")
            nc.sync.dma_start(fcw[:], fcw_ext[:])
            eye = cpool.tile([128, 128], mdt, name="sb_eye")
            nc.sync.dma_start(eye[:], eye_ext[:])

            h_buf = []
            c_buf = []
            for l in range(NLAYERS):
                ht = wpool.tile([128, KT * B], mdt, tag=f"h{l}", bufs=3,
                                name=f"h{l}_init")
                nc.sync.dma_start(ht[:], hinit_ext[l][:])
                h_buf.append(ht)
                ct = wpool.tile([HC, B], f32, tag=f"c{l}", name=f"c{l}_init")
                nc.sync.dma_start(ct[:], cinit_ext[l][:])
                c_buf.append(ct)

            w_hh = {0: w_sb["whh0"], 1: w_sb["whh1"], 2: w_sb["whh2"]}
            w_ih = {1: w_sb["wih1"], 2: w_sb["wih2"]}

            def emit_layer(l, p):
                """LSTM layer l consuming h_buf/c_buf state; returns AG out."""
                h_self = h_buf[l]
                h_below = h_buf[l - 1] if l > 0 else None
                # gate order in weights: i,f,g,o ; emit f,i,g,o so the cell
                # can start as early as possible.
                acts = {}

                def emit_gate(g):
                    ps = ppool.tile([HC, B], f32, tag="gates", bufs=6,
                                    name=f"ps_l{l}_p{p}_g{g}")
                    n_mm = (1 if l == 0 else 0) + KT * (1 if l == 0 else 2)
                    idx = 0
                    if l == 0:
                        nc.tensor.matmul(ps[:], eye[:],
                                         gx0[:, g * B:(g + 1) * B],
                                         start=True, stop=(idx == n_mm - 1))
                        idx += 1
                    for kt in range(KT):
                        nc.tensor.matmul(
                            ps[:],
                            w_hh[l][:, (kt * 4 + g) * HC:(kt * 4 + g + 1) * HC],
                            h_self[:, kt * B:(kt + 1) * B],
                            start=(idx == 0), stop=(idx == n_mm - 1))
                        idx += 1
                    if l > 0:
                        for kt in range(KT):
                            nc.tensor.matmul(
                                ps[:],
                                w_ih[l][:, (kt * 4 + g) * HC:(kt * 4 + g + 1) * HC],
                                h_below[:, kt * B:(kt + 1) * B],
                                start=False, stop=(idx == n_mm - 1))
                            idx += 1
                    a = wpool.tile([HC, B], f32, tag="gact", bufs=10,
                                   name=f"act_l{l}_p{p}_g{g}")
                    func = AF.Tanh if g == 2 else AF.Sigmoid
                    bias = b_sb[l][:, g:g + 1] if l > 0 else 0.0
                    nc.scalar.activation(a[:], ps[:], func, bias=bias)
                    acts[g] = a

                # f, i, g gates first; c2 + tanh(c2) emitted BEFORE the o
                # gate's activation so tanh(c2) precedes it in the ACT queue
                # (it is ready earlier) — hch then waits only on o's act.
                for g in (1, 0, 2):
                    emit_gate(g)
                t_fc = wpool.tile([HC, B], f32, tag="tfc", bufs=3, name=f"tfc{l}_{p}")
                nc.vector.tensor_mul(t_fc[:], acts[1][:], c_buf[l][:])
                t_ig = wpool.tile([HC, B], f32, tag="tig", bufs=3, name=f"tig{l}_{p}")
                nc.vector.tensor_mul(t_ig[:], acts[0][:], acts[2][:])
                c_new = wpool.tile([HC, B], f32, tag=f"c{l}", name=f"c{l}_p{p}")
                nc.vector.tensor_add(c_new[:], t_fc[:], t_ig[:])
                c_buf[l] = c_new
                th = wpool.tile([HC, B], f32, tag="th", bufs=3, name=f"th{l}_{p}")
                nc.scalar.activation(th[:], c_new[:], AF.Tanh)
                emit_gate(3)
                hch = wpool.tile([HC, B], mdt, tag="hch", bufs=6, name=f"hch{l}_{p}")
                nc.vector.tensor_mul(hch[:], acts[3][:], th[:])
                return hch

            def emit_fc(t, hch2):
                # K-sharded fc: partial = fc_W[128k:128k+128, :].T @ h2_chunk
                # (local, no gathered h2 needed); host sums the 8 partials.
                ps1 = ppool.tile([128, B], f32, tag="fc1", bufs=1,
                                 name=f"psfc1_{t}")
                nc.tensor.matmul(ps1[:], fcw[:, 0:128], hch2[:],
                                 start=True, stop=True)
                ps2 = ppool.tile([FCP - 128, B], f32, tag="fc2", bufs=1,
                                 name=f"psfc2_{t}")
                nc.tensor.matmul(ps2[:], fcw[:, 128:FCP], hch2[:],
                                 start=True, stop=True)
                o1 = wpool.tile([128, B], mdt, tag="osb1", bufs=2, name=f"o1_{t}")
                nc.vector.tensor_copy(o1[:], ps1[:])
                o2 = wpool.tile([FCP - 128, B], mdt, tag="osb2", bufs=2,
                                name=f"o2_{t}")
                nc.vector.tensor_copy(o2[:], ps2[:])
                # out stores go on the gpsimd SWDGE queue: the sync queue is
                # reserved for AG-landing DMAs (the phase-critical path).
                nc.gpsimd.dma_start(out_ext[t, 0:128, :], o1[:])
                nc.gpsimd.dma_start(out_ext[t, 128:FCP, :], o2[:])

            def emit_ag(layers_hch, p):
                """AllGather the given layers' chunks (merged when >1)."""
                nl = len(layers_hch)
                tag = "ag" + "".join(str(l) for l in layers_hch)
                agi = dpool.tile([nl * HC, B], mdt, tag=f"i{tag}",
                                 name=f"agi{tag}_p{p}")
                for i, (l, hch) in enumerate(layers_hch.items()):
                    # staging on the scalar HWDGE queue: fast descriptor gen,
                    # and it lands right after this layer's activations there;
                    # sync stays landing-only, gpsimd carries trigger + outs
                    nc.scalar.dma_start(agi[i * HC:(i + 1) * HC, :], hch[:])
                ago = dpool.tile([nl * H, B], mdt, tag=f"o{tag}",
                                 addr_space="Shared", name=f"ago{tag}_p{p}")
                nc.gpsimd.collective_compute(
                    "AllGather", mybir.AluOpType.bypass, replica_groups=rg,
                    ins=[agi[:].opt()], outs=[ago[:].opt()])
                # land gathers into fresh SBUF h tiles on the sync HWDGE
                # queue, which carries ONLY landings: the wait-for-AG then
                # never head-of-line blocks activations (scalar) or bounce
                # DMAs (gpsimd), and the landing issues the moment the AG
                # completes — it is the phase-critical chain.
                ago_v = ago.rearrange("(j l p) f -> l p j f", l=nl, p=HC)
                new_h = {}
                for i, l in enumerate(layers_hch):
                    h_new = wpool.tile([128, KT * B], mdt, tag=f"h{l}", bufs=3,
                                       name=f"h{l}_p{p}")
                    nc.sync.dma_start(
                        h_new.rearrange("p (j f) -> p j f", j=KT), ago_v[i])
                    new_h[l] = h_new
                # NOTE: caller installs new_h into h_buf only after ALL of
                # this phase's emit_layer calls — installing it here made
                # layer l+1 consume the CURRENT phase's gather (off-by-one
                # step, rel err 3.3e-2, and serialized l+1 behind the AG).
                return new_h

            # ---- wavefront over phases -----------------------------------
            # Emission order is load-bearing: each layer's AG staging +
            # doorbell is emitted IMMEDIATELY after its cell, and the fc
            # copies/out-stores come LAST.  Emitting fc before the AGs put
            # the out DMAs ahead of the doorbells on the gpsimd queue, whose
            # wait-for-fc-copy then delayed ALL the phase's AG triggers to
            # the end of the compute (~12us late, traced 2026-08-08) — that
            # ordering bug, not collective speed, was the 35us phase.
            # (Merging l0+l1 into one AG was also tried: 128KB/rank merged
            # collective ran ~12us active vs ~2x5us solo — keep solo AGs.)
            for p in range(steps + NLAYERS - 1):
                new_h = {}
                if p <= steps - 1:
                    new_h.update(emit_ag({0: emit_layer(0, p)}, p))
                if 1 <= p <= steps:
                    new_h.update(emit_ag({1: emit_layer(1, p)}, p))
                if 2 <= p <= steps + 1:
                    hch2 = emit_layer(2, p)
                    new_h.update(emit_ag({2: hch2}, p))
                    emit_fc(p - 2, hch2)
                for l, t in new_h.items():
                    h_buf[l] = t

    nc.compile()
    return nc


def _get_nc(steps: int):
    key = (steps, MM_DTYPE)
    if key not in _BUILD_CACHE:
        _BUILD_CACHE[key] = _build(steps)
    return _BUILD_CACHE[key]


def _prep_inputs(inputs, W_ih0, W_hh0, b_ih0, b_hh0, W_ih1, W_hh1, b_ih1, b_hh1,
                 W_ih2, W_hh2, b_ih2, b_hh2, fc_W, fc_b, inh_W, inh_b,
                 inc_W, inc_b, labels):
    """Host-side shard prep: all arrays already in the on-device layout."""
    f32 = np.float32
    frame = np.asarray(inputs, f32).reshape(B, OUT)
    onehot = np.zeros((B, NUM_CLASSES), f32)
    onehot[:, int(labels)] = 1.0
    x_in = np.concatenate([frame, onehot], axis=1)                  # [B, 216]

    h0_all = frame @ np.asarray(inh_W, f32) + np.asarray(inh_b, f32)  # [B, 3H]
    c0_all = frame @ np.asarray(inc_W, f32) + np.asarray(inc_b, f32)
    gx0_full = (x_in @ np.asarray(W_ih0, f32)
                + np.asarray(b_ih0, f32) + np.asarray(b_hh0, f32))  # [B, 4H]

    fcw_pad = np.zeros((H, FCP), f32)
    fcw_pad[:, :OUT] = np.asarray(fc_W, f32)
    fcb_pad = np.zeros(FCP, f32)
    fcb_pad[:OUT] = np.asarray(fc_b, f32)

    def pack_w(Wfull, k):
        Wfull = np.asarray(Wfull, f32)
        cols = np.concatenate(
            [Wfull[:, g * H + k * HC: g * H + (k + 1) * HC] for g in range(4)],
            axis=1)                                                  # [K, 512]
        kt = Wfull.shape[0] // 128
        return np.ascontiguousarray(
            cols.reshape(kt, 128, 4, HC).transpose(1, 0, 2, 3).reshape(128, kt * GC))

    def pack_bias(bi, bh, k):
        s = np.asarray(bi, f32) + np.asarray(bh, f32)
        return np.ascontiguousarray(
            np.stack([s[g * H + k * HC: g * H + (k + 1) * HC] for g in range(4)],
                     axis=1))                                        # [128, 4]

    def pack_hT(h_l):   # [B, H] -> [128, KT*B]
        hT = np.ascontiguousarray(h_l.T)                             # [H, B]
        return np.ascontiguousarray(
            hT.reshape(KT, 128, B).transpose(1, 0, 2).reshape(128, KT * B))

    if MM_DTYPE == "bfloat16":
        import ml_dtypes
        mnp = ml_dtypes.bfloat16
    else:
        mnp = np.float32

    def mcast(a):
        return np.ascontiguousarray(a.astype(mnp))

    eye = mcast(np.eye(128, dtype=f32))
    in_maps = []
    for k in range(NCORES):
        m = {"eye": eye}
        m["whh0"] = mcast(pack_w(W_hh0, k))
        m["wih1"] = mcast(pack_w(W_ih1, k))
        m["whh1"] = mcast(pack_w(W_hh1, k))
        m["wih2"] = mcast(pack_w(W_ih2, k))
        m["whh2"] = mcast(pack_w(W_hh2, k))
        m["b1"] = pack_bias(b_ih1, b_hh1, k)
        m["b2"] = pack_bias(b_ih2, b_hh2, k)
        gx = np.stack(
            [gx0_full[:, g * H + k * HC: g * H + (k + 1) * HC].T for g in range(4)],
            axis=1)                                                  # [128, 4, B]
        m["gx0"] = mcast(gx.reshape(128, 4 * B))
        m["fcw"] = mcast(fcw_pad[k * 128:(k + 1) * 128, :])          # [128, 208]
        for l in range(NLAYERS):
            m[f"h{l}i"] = mcast(pack_hT(h0_all[:, l * H:(l + 1) * H]))
            m[f"c{l}i"] = np.ascontiguousarray(
                c0_all[:, l * H:(l + 1) * H].T[k * HC:(k + 1) * HC, :])
        in_maps.append(m)
    return in_maps, fcb_pad


def _run(steps, in_maps, trace=False):
    from concourse import bass_utils
    nc = _get_nc(steps)
    return bass_utils.run_bass_kernel_spmd(
        nc, in_maps, core_ids=list(range(NCORES)), trace=trace)


def _assemble(results, steps, fcb_pad):
    # per-core K-shard partials [steps, FCP, B]: sum + bias -> [B, steps, 68, 3]
    full = results[0]["out"].astype(np.float64)
    for k in range(1, NCORES):
        full += results[k]["out"]
    full = full.astype(np.float32) + fcb_pad[None, :, None]
    full = full.transpose(2, 0, 1)[:, :, :OUT]       # [B, steps, 204]
    return np.ascontiguousarray(full.reshape(B, steps, 68, 3).astype(np.float32))


def kernel(**inputs) -> np.ndarray:
    in_maps, fcb_pad = _prep_inputs(**inputs)
    last_err = None
    for attempt in range(3):
        try:
            res = _run(STEPS, in_maps, trace=False)
            return _assemble(res.results, STEPS, fcb_pad)
        except Exception as e:  # transient NRT device-unrecoverable errors
            last_err = e
    raise last_err

